# revision 18
# baseline (speedup 1.0000x reference)
"""Trainium2 Bass kernel: sliding-window GQA attention block.

Computation (matches the PyTorch/JAX reference):
    q,k,v = x @ {Wq,Wk,Wv}.T ; QK-RMSNorm ; RoPE ; GQA repeat(4x) ;
    softmax(q k^T / sqrt(D) + sliding-window bias(|i-j|<=512)) v ; @ Wo.T

Sharding (no collectives): 8 cores = 2 batches x 4 sequence chunks of 512
tokens.  Each core computes its 512 own tokens for ALL 16 heads, using a
512-token halo either side for K/V (halo K/V recomputed locally), then the
full o_proj rows for its tokens.  Outputs concatenate on host.

Precision/speed split:
  * Q/K/V projections: fp8(e4m3) DoubleRow matmuls (0.5 cyc/row, 256-deep
    contraction) with the activation split hi+lo into TWO fp8 terms
    (x = x8 + xr) accumulated in PSUM; weights single fp8 scaled by 64
    (RMSNorm makes Q/K scale-invariant; V's 64x rides through attention).
    Net cost 2 DR units = half of fp32r, with only the W-quantization
    error (~the fp8 step of W) remaining.
  * Attention: bf16 operands (kT/qT/v/e), full-rate 1 cyc/row at any
    moving size, which enables diagonal 128-query tiling: per (head,
    128-q tile) exactly 9 key tiles are live; only tiles 0 and 8 touch
    the window edge and take a constant triangle mask; ONE grouped exp
    activation covers all 9 score tiles [128, 1152].
  * Sequence-edge (first/last core chunks): K/V and RoPE tables are
    zero-padded, so out-of-sequence keys contribute exp(0)=1 to the
    softmax denominator; a host-computed count is subtracted from the
    denominator via a tiny extra PSUM-accumulate matmul.
  * o_proj: bf16 aoT x bf16 Wo (1 cyc/row).

Layouts: projections contract over hidden with hidden on partitions; Q/K
land as [head_dim, tokens] so scores^T and PV need no transposes; RMSNorm
partition reductions via ones-matmuls; softmax denominators via
ones-matmuls accumulated alongside PV; normalization applied post-PV via
a PE broadcast of reciprocal denominators.  Head dims are host-interleaved
[0,64,1,65,...] so RoPE's rotate_half is a single DVE stream-shuffle.
"""

import numpy as np


def _ensure_path():
    try:
        import concourse  # noqa: F401
    except ImportError:
        import sys
        for p in ("/opt/trn_rl_repo", "/root/.axon_site/_ro/trn_rl_repo"):
            if p not in sys.path:
                sys.path.insert(0, p)


H, KV, D = 16, 4, 128
GQ = H // KV            # 4 query heads per kv head
WIN = 512
EPS = 1e-6
B, L, HID = 2, 2048, 2048
OWN = 512               # tokens owned per core
HALO = 1536             # key/value token window per core (own +- 512)
NKT = HALO // 128       # 12 key tiles of 128
NHK = HID // 128        # 16 contraction tiles over hidden
NHP = NHK // 2          # 8 hidden-pair tiles (256-deep fp8 DoubleRow)
NQT = OWN // 128        # 4 diagonal query tiles of 128
NKD = 9                 # key tiles per 128-query window (1024+128)/128
N_CORES = 8
WSCALE = 64.0           # host-side fp8 weight scale
NEG = -1.0e30

_CACHE = {}


def _build():
    _ensure_path()
    import concourse.mybir as mybir
    import concourse.tile as tile
    from concourse import bacc
    from contextlib import ExitStack

    F32 = mybir.dt.float32
    F32R = mybir.dt.float32r
    F8 = mybir.dt.float8e4
    BF = mybir.dt.bfloat16
    DR = mybir.MatmulPerfMode.DoubleRow
    ACTF = mybir.ActivationFunctionType

    nc = bacc.Bacc("TRN2", target_bir_lowering=False, debug=False,
                   num_devices=N_CORES)

    x8T = nc.dram_tensor("x8T", [HID, HALO], F8, kind="ExternalInput").ap()
    xrT = nc.dram_tensor("xrT", [HID, HALO], F8, kind="ExternalInput").ap()
    WqT = nc.dram_tensor("WqT", [HID, H * D], F8, kind="ExternalInput").ap()
    WkT = nc.dram_tensor("WkT", [HID, KV * D], F8, kind="ExternalInput").ap()
    WvT = nc.dram_tensor("WvT", [HID, KV * D], F8, kind="ExternalInput").ap()
    WoT = nc.dram_tensor("WoT", [H * D, HID], BF, kind="ExternalInput").ap()
    # RoPE tables, transposed to [D, tokens], norm-weights (and for q the
    # 1/sqrt(D) score scale) folded in; s-table has rotate_half sign/roll.
    cqT = nc.dram_tensor("cqT", [D, OWN], F32, kind="ExternalInput").ap()
    sqT = nc.dram_tensor("sqT", [D, OWN], F32, kind="ExternalInput").ap()
    ckT = nc.dram_tensor("ckT", [D, HALO], F32, kind="ExternalInput").ap()
    skT = nc.dram_tensor("skT", [D, HALO], F32, kind="ExternalInput").ap()
    # [128, 2, 128] window-edge triangle masks (slot 0 -> kt0, 1 -> kt8)
    band = nc.dram_tensor("band", [128, 2, 128], F32,
                          kind="ExternalInput").ap()
    # negated out-of-sequence key counts per (q-tile, query)
    dcorr = nc.dram_tensor("dcorr", [1, NQT * 128], BF,
                           kind="ExternalInput").ap()
    out = nc.dram_tensor("out", [OWN, HID], F32, kind="ExternalOutput").ap()

    SWAP_MASK = [p ^ 1 for p in range(32)]

    with tile.TileContext(nc) as tc, ExitStack() as top:
        # ---- persistent SBUF ----
        keep = top.enter_context(tc.tile_pool(name="keep", bufs=1))
        v_sb = keep.tile([128, NKT, KV * D], BF)        # [tok128, ktile, vf]
        kT_sb = keep.tile([128, KV, HALO], BF)          # [d, kv, tok]
        qT_sb = keep.tile([128, H, OWN], BF)            # [d, h, tok]
        ones32 = keep.tile([128, 1], F32)
        nc.vector.memset(ones32, 1.0)
        ones_sb = keep.tile([128, 1], F32R)
        nc.vector.tensor_copy(ones_sb, ones32)
        ones_bf = keep.tile([128, 1], BF)
        nc.vector.tensor_copy(ones_bf, ones32)
        one1_bf = keep.tile([1, 1], BF)
        nc.vector.memset(one1_bf, 1.0)
        ones132 = keep.tile([1, 128], F32)
        nc.vector.memset(ones132, 1.0)
        ones1_sb = keep.tile([1, 128], F32R)
        nc.vector.tensor_copy(ones1_sb, ones132)
        band_sb = keep.tile([128, 2, 128], F32)
        nc.gpsimd.dma_start(out=band_sb, in_=band)
        dcorr_sb = keep.tile([1, NQT * 128], BF)
        nc.gpsimd.dma_start(out=dcorr_sb, in_=dcorr)

        # alternating resident zones: a phase's tensors prefetch while the
        # *other* zone's previous-phase readers drain
        zoneA = top.enter_context(tc.tile_pool(name="zoneA", bufs=1))
        zoneB = top.enter_context(tc.tile_pool(name="zoneB", bufs=1))
        xs = top.enter_context(tc.tile_pool(name="xs", bufs=18))
        ws = top.enter_context(tc.tile_pool(name="ws", bufs=10))

        def load_paired(dst, src, n):
            # [256r, C] DRAM rows -> [128, 2, C] fp8 tiles (k = 2p+s packing;
            # any bijection works since x and W tiles load identically)
            for k in range(n):
                nc.gpsimd.dma_start(out=dst[:, k],
                                    in_=src[k * 256:(k + 1) * 256, :])

        def load_x_terms(ch_lo, width):
            """Stream both x hi/lo fp8 tiles for a token slice."""
            tiles = []
            for src in (x8T, xrT):
                cur = []
                for k in range(NHP):
                    xt = xs.tile([128, 2, width], F8, tag="xt")
                    nc.sync.dma_start(
                        out=xt, in_=src[k * 256:(k + 1) * 256,
                                        ch_lo:ch_lo + width])
                    cur.append(xt)
                tiles.append(cur)
            return tiles

        # ================= V projection (2-term fp8 DR) =================
        wv_sb = zoneA.tile([128, NHP, 2, KV * D], F8, tag="wv", name="wv_sb")
        load_paired(wv_sb, WvT, NHP)
        with ExitStack() as ph:
            ps = ph.enter_context(tc.tile_pool(name="vps", bufs=8,
                                               space="PSUM"))
            for ch in range(3):
                pv = [ps.tile([128, KV * D], F32, tag="pv", name=f"pv{t}")
                      for t in range(4)]
                terms = load_x_terms(ch * 512, 512)
                for fh in range(2):
                    for t, xts in enumerate(terms):
                        for k in range(NHP):
                            for tt in range(4):
                                nc.tensor.matmul(
                                    pv[tt][:, fh * 256:(fh + 1) * 256],
                                    xts[k][:, :, tt * 128:(tt + 1) * 128],
                                    wv_sb[:, k, :, fh * 256:(fh + 1) * 256],
                                    start=(t == 0 and k == 0),
                                    stop=(t == 1 and k == NHP - 1),
                                    perf_mode=DR)
                for tt in range(4):
                    nc.scalar.copy(out=v_sb[:, ch * 4 + tt, :], in_=pv[tt])

        # ============ K / Q projection + RMSNorm + RoPE ============
        def norm_rope(p_feat, cT, sT, r_dst, n_tok, psn, scratch):
            """p_feat: psum [128 d, n_tok] raw head; writes r_dst (bf16)."""
            sq = scratch.tile([128, n_tok], F32R, tag="sq")
            nc.vector.tensor_mul(out=sq, in0=p_feat, in1=p_feat)
            pss = psn.tile([1, n_tok], F32, tag="ss")
            nc.tensor.matmul(pss, ones_sb, sq, start=True, stop=True)
            ms = scratch.tile([1, n_tok], F32, tag="ms")
            nc.vector.tensor_scalar(out=ms, in0=pss, scalar1=1.0 / D,
                                    scalar2=EPS * WSCALE * WSCALE,
                                    op0=mybir.AluOpType.mult,
                                    op1=mybir.AluOpType.add)
            nc.vector.reciprocal(ms, ms)
            rs = scratch.tile([1, n_tok], F32R, tag="rs")
            nc.scalar.activation(out=rs, in_=ms, func=ACTF.Sqrt)
            prb = psn.tile([128, n_tok], F32, tag="rb")
            nc.tensor.matmul(prb, ones1_sb, rs, start=True, stop=True)
            swp = scratch.tile([128, n_tok], F32, tag="swp")
            nc.vector.stream_shuffle(out=swp, in_=p_feat, mask=SWAP_MASK)
            t1 = scratch.tile([128, n_tok], F32, tag="t1")
            nc.gpsimd.tensor_mul(out=t1, in0=p_feat, in1=cT)
            t2 = scratch.tile([128, n_tok], F32, tag="t2")
            nc.gpsimd.tensor_mul(out=t2, in0=swp, in1=sT)
            nc.gpsimd.tensor_add(out=t1, in0=t1, in1=t2)
            nc.vector.tensor_mul(out=r_dst, in0=t1, in1=prb)

        wk_sb = zoneB.tile([128, NHP, 2, KV * D], F8, tag="wk", name="wk_sb")
        load_paired(wk_sb, WkT, NHP)
        ck_sb = zoneB.tile([128, HALO], F32, tag="tc", name="ck_sb")
        sk_sb = zoneB.tile([128, HALO], F32, tag="ts", name="sk_sb")
        nc.gpsimd.dma_start(out=ck_sb, in_=ckT)
        nc.gpsimd.dma_start(out=sk_sb, in_=skT)

        with ExitStack() as ph:
            psp = ph.enter_context(tc.tile_pool(name="psp", bufs=6,
                                                space="PSUM"))
            psn = ph.enter_context(tc.tile_pool(name="psn", bufs=1,
                                                space="PSUM"))
            scratch = ph.enter_context(tc.tile_pool(name="scratch", bufs=2))
            for ch in range(3):
                pk = [psp.tile([128, 512], F32, tag="p", name=f"pk{t}")
                      for t in range(KV)]
                terms = load_x_terms(ch * 512, 512)
                for th in range(2):
                    for t, xts in enumerate(terms):
                        for k in range(NHP):
                            for h in range(KV):
                                nc.tensor.matmul(
                                    pk[h][:, th * 256:(th + 1) * 256],
                                    wk_sb[:, k, :, h * 128:(h + 1) * 128],
                                    xts[k][:, :, th * 256:(th + 1) * 256],
                                    start=(t == 0 and k == 0),
                                    stop=(t == 1 and k == NHP - 1),
                                    perf_mode=DR)
                sl = slice(ch * 512, (ch + 1) * 512)
                for h in range(KV):
                    norm_rope(pk[h], ck_sb[:, sl], sk_sb[:, sl],
                              kT_sb[:, h, sl], 512, psn, scratch)

            # ---- Q: own-x terms resident in zone A (Wv readers done) ----
            xo8_sb = zoneA.tile([128, NHP, 2, OWN], F8, tag="xo8",
                                name="xo8_sb")
            xor_sb = zoneA.tile([128, NHP, 2, OWN], F8, tag="xor",
                                name="xor_sb")
            load_paired(xo8_sb, x8T[:, 512:1024], NHP)
            load_paired(xor_sb, xrT[:, 512:1024], NHP)
            cq_sb = zoneA.tile([128, OWN], F32, tag="tc", name="cq_sb")
            sq_sb = zoneA.tile([128, OWN], F32, tag="ts", name="sq_sb")
            nc.gpsimd.dma_start(out=cq_sb, in_=cqT)
            nc.gpsimd.dma_start(out=sq_sb, in_=sqT)
            for qf in range(4):
                pq = [psp.tile([128, OWN], F32, tag="p", name=f"pq{t}")
                      for t in range(4)]
                wqs = []
                for k in range(NHP):
                    wq = ws.tile([128, 2, 512], F8, tag="w")
                    nc.sync.dma_start(
                        out=wq, in_=WqT[k * 256:(k + 1) * 256,
                                        qf * 512:(qf + 1) * 512])
                    wqs.append(wq)
                for th in range(2):
                    for t, xo in enumerate((xo8_sb, xor_sb)):
                        for k in range(NHP):
                            for j in range(4):
                                nc.tensor.matmul(
                                    pq[j][:, th * 256:(th + 1) * 256],
                                    wqs[k][:, :, j * 128:(j + 1) * 128],
                                    xo[:, k, :, th * 256:(th + 1) * 256],
                                    start=(t == 0 and k == 0),
                                    stop=(t == 1 and k == NHP - 1),
                                    perf_mode=DR)
                for j in range(4):
                    norm_rope(pq[j], cq_sb, sq_sb, qT_sb[:, qf * 4 + j, :],
                              OWN, psn, scratch)

        # ================= attention (bf16, diagonal 128-q tiles) ========
        aoT_sb = zoneA.tile([128, H, OWN], BF, tag="ao", name="aoT_sb")
        with ExitStack() as ph:
            pssc = ph.enter_context(tc.tile_pool(name="aps", bufs=1,
                                                 space="PSUM"))
            pso = ph.enter_context(tc.tile_pool(name="apo", bufs=2,
                                                space="PSUM"))
            psd = ph.enter_context(tc.tile_pool(name="apd", bufs=2,
                                                space="PSUM"))
            psb = ph.enter_context(tc.tile_pool(name="apb", bufs=1,
                                                space="PSUM"))
            es = ph.enter_context(tc.tile_pool(name="aes", bufs=3))
            sc = ph.enter_context(tc.tile_pool(name="asc", bufs=2))
            for h in range(H):
                kv = h // GQ
                po_h = pso.tile([128, OWN], F32, tag="po")
                pd_h = psd.tile([1, OWN], F32, tag="pd")
                for qt in range(NQT):
                    qsl = slice(qt * 128, (qt + 1) * 128)
                    ps_s = pssc.tile([128, NKD, 128], F32, tag="s")
                    for j in range(NKD):
                        nc.tensor.matmul(
                            ps_s[:, j, :],
                            kT_sb[:, kv, (qt + j) * 128:(qt + j + 1) * 128],
                            qT_sb[:, h, qsl], start=True, stop=True)
                    nc.vector.tensor_add(out=ps_s[:, 0::8, :],
                                         in0=ps_s[:, 0::8, :], in1=band_sb)
                    e = es.tile([128, NKD, 128], BF, tag="e")
                    nc.scalar.activation(out=e, in_=ps_s, func=ACTF.Exp)
                    for j in range(NKD):
                        nc.tensor.matmul(po_h[:, qsl],
                                         v_sb[:, qt + j,
                                              kv * 128:(kv + 1) * 128],
                                         e[:, j, :], start=(j == 0),
                                         stop=(j == NKD - 1),
                                         skip_group_check=True)
                        nc.tensor.matmul(pd_h[:, qsl], ones_bf, e[:, j, :],
                                         start=(j == 0), stop=False,
                                         skip_group_check=True)
                    nc.tensor.matmul(pd_h[:, qsl], one1_bf, dcorr_sb[:, qsl],
                                     start=False, stop=True,
                                     skip_group_check=True)
                dr32 = sc.tile([1, OWN], F32, tag="dr32")
                nc.vector.reciprocal(dr32, pd_h)
                drr = sc.tile([1, OWN], F32R, tag="drr")
                nc.vector.tensor_copy(drr, dr32)
                pb = psb.tile([128, OWN], F32, tag="pb")
                nc.tensor.matmul(pb, ones1_sb, drr, start=True, stop=True)
                nc.vector.tensor_mul(out=aoT_sb[:, h, :], in0=po_h, in1=pb)

        # ================= output projection (bf16) =================
        with ExitStack() as ph:
            psy = ph.enter_context(tc.tile_pool(name="ops", bufs=8,
                                                space="PSUM"))
            ys = ph.enter_context(tc.tile_pool(name="oy", bufs=4))
            for hc in range(4):
                py = [psy.tile([128, 512], F32, tag="py", name=f"py{t}")
                      for t in range(4)]
                for h in range(H):
                    wo = ws.tile([128, 512], BF, tag="w")
                    eng = nc.sync if h % 2 == 0 else nc.scalar
                    eng.dma_start(
                        out=wo, in_=WoT[h * 128:(h + 1) * 128,
                                        hc * 512:(hc + 1) * 512])
                    for tt in range(4):
                        nc.tensor.matmul(py[tt],
                                         aoT_sb[:, h, tt * 128:(tt + 1) * 128],
                                         wo, start=(h == 0), stop=(h == H - 1))
                for tt in range(4):
                    y = ys.tile([128, 512], F32, tag="y")
                    nc.vector.tensor_scalar(out=y, in0=py[tt],
                                            scalar1=1.0 / WSCALE, scalar2=0.0,
                                            op0=mybir.AluOpType.mult,
                                            op1=mybir.AluOpType.add)
                    nc.sync.dma_start(
                        out=out[tt * 128:(tt + 1) * 128,
                                hc * 512:(hc + 1) * 512], in_=y)

    nc.compile()
    return nc


def _host_prep(x, cos, sin, Wq, Wk, Wv, Wo, q_norm_w, k_norm_w):
    """Build the 8 per-core input dicts."""
    import ml_dtypes
    f8 = ml_dtypes.float8_e4m3
    bf16 = ml_dtypes.bfloat16
    scale = 1.0 / np.sqrt(D)
    # interleave head dims [0,64,1,65,...]: rotate_half partners end up on
    # adjacent partitions so the kernel swaps them with one stream-shuffle
    perm = np.empty(D, np.int64)
    perm[0::2] = np.arange(64)
    perm[1::2] = 64 + np.arange(64)

    def rope_tables(cos_r, sin_r, w, extra):
        # fold norm weight (and any extra scale); sign/roll for rotate_half
        c = (cos_r * w[None, :] * extra).astype(np.float32)
        w_rot = np.roll(w, -64)
        s = (sin_r * w_rot[None, :] * extra).astype(np.float32)
        s[:, :64] *= -1.0
        return (np.ascontiguousarray(c.T[perm]),
                np.ascontiguousarray(s.T[perm]))

    idx_q = (np.arange(H)[:, None] * D + perm[None, :]).ravel()
    idx_k = (np.arange(KV)[:, None] * D + perm[None, :]).ravel()
    WqT = np.ascontiguousarray(Wq.T[:, idx_q] * WSCALE).astype(f8)
    WkT = np.ascontiguousarray(Wk.T[:, idx_k] * WSCALE).astype(f8)
    WvT = np.ascontiguousarray(Wv.T * WSCALE).astype(f8)
    WoT = np.ascontiguousarray(Wo.T).astype(bf16)

    # window-edge triangle masks for key tiles 0 and 8 of each 128-q window
    jj = np.arange(128)[:, None]
    ii = np.arange(128)[None, :]
    band = np.zeros((128, 2, 128), np.float32)
    band[:, 0, :] = np.where(jj < ii, NEG, 0.0)   # kt0: key - query < -512
    band[:, 1, :] = np.where(jj > ii, NEG, 0.0)   # kt8: key - query > +512

    in_maps = []
    for c in range(N_CORES):
        b, ch = divmod(c, 4)
        start = ch * OWN
        lo, hi = start - WIN, start + OWN + WIN
        vlo, vhi = max(lo, 0), min(hi, L)
        xh = np.zeros((HALO, HID), np.float32)
        xh[vlo - lo:vhi - lo] = x[b, vlo:vhi]
        ch_cos = np.zeros((HALO, D), np.float32)
        ch_sin = np.zeros((HALO, D), np.float32)
        ch_cos[vlo - lo:vhi - lo] = cos[vlo:vhi]
        ch_sin[vlo - lo:vhi - lo] = sin[vlo:vhi]
        ckT, skT = rope_tables(ch_cos, ch_sin, k_norm_w, 1.0)
        cqT, sqT = rope_tables(cos[start:start + OWN], sin[start:start + OWN],
                               q_norm_w, scale)
        # hi/lo fp8 split of the (transposed) halo activations
        xhT = np.ascontiguousarray(xh.T)
        x8 = xhT.astype(f8)
        xr = (xhT - x8.astype(np.float32)).astype(f8)
        # negated count of out-of-sequence keys reaching the softmax
        # denominator as exp(0)=1 (zero-padded K => score 0, unless the
        # window-edge triangle already masks that (j, i) entry)
        dcorr = np.zeros((NQT, 128), np.float32)
        for qt in range(NQT):
            q0g = start + qt * 128
            kk = q0g - WIN + np.arange(NKD * 128)
            invalid = (kk < 0) | (kk >= L)          # [1152]
            inv = invalid.reshape(NKD, 128)         # [kt, j]
            for i in range(128):
                cnt = inv.sum()
                cnt -= inv[0, :][jj[:, 0] < i].sum()   # kt0 triangle masked
                cnt -= inv[8, :][jj[:, 0] > i].sum()   # kt8 triangle masked
                dcorr[qt, i] = -float(cnt)
        in_maps.append({
            "x8T": x8, "xrT": xr,
            "WqT": WqT, "WkT": WkT, "WvT": WvT, "WoT": WoT,
            "cqT": cqT, "sqT": sqT, "ckT": ckT, "skT": skT,
            "band": band, "dcorr": dcorr.reshape(1, NQT * 128).astype(bf16),
        })
    return in_maps


def kernel(**inputs):
    _ensure_path()
    from concourse import bass_utils

    if "nc" not in _CACHE:
        _CACHE["nc"] = _build()
    nc = _CACHE["nc"]

    in_maps = _host_prep(
        np.asarray(inputs["x"]), np.asarray(inputs["cos"]),
        np.asarray(inputs["sin"]), np.asarray(inputs["Wq"]),
        np.asarray(inputs["Wk"]), np.asarray(inputs["Wv"]),
        np.asarray(inputs["Wo"]), np.asarray(inputs["q_norm_w"]),
        np.asarray(inputs["k_norm_w"]))

    res = bass_utils.run_bass_kernel_spmd(nc, in_maps,
                                          core_ids=list(range(N_CORES)))
    out = np.empty((B, L, HID), np.float32)
    for c in range(N_CORES):
        b, ch = divmod(c, 4)
        out[b, ch * OWN:(ch + 1) * OWN] = res.results[c]["out"]
    return out


# revision 31
# speedup vs baseline: 1.0542x; 1.0542x over previous
"""Trainium2 Bass kernel: sliding-window GQA attention block.

Computation (matches the PyTorch/JAX reference):
    q,k,v = x @ {Wq,Wk,Wv}.T ; QK-RMSNorm ; RoPE ; GQA repeat(4x) ;
    softmax(q k^T / sqrt(D) + sliding-window bias(|i-j|<=512)) v ; @ Wo.T

Sharding (no collectives): 8 cores = 2 batches x 4 sequence chunks of 512
tokens.  Each core computes its 512 own tokens for ALL 16 heads, using a
512-token halo either side for K/V (halo K/V recomputed locally), then the
full o_proj rows for its tokens.  Outputs concatenate on host.

Precision/speed split:
  * Q/K/V projections: fp8(e4m3) DoubleRow matmuls (0.5 cyc/row, 256-deep
    contraction) with the activation split hi+lo into TWO fp8 terms
    (x = x8 + xr) accumulated in PSUM; weights single fp8 scaled by 64
    (RMSNorm makes Q/K scale-invariant; V's 64x rides through attention).
    Net cost 2 DR units = half of fp32r, with only the W-quantization
    error (~the fp8 step of W) remaining.
  * Attention: bf16 operands (kT/qT/v/e), full-rate 1 cyc/row at any
    moving size, which enables diagonal 128-query tiling: per (head,
    128-q tile) exactly 9 key tiles are live; only tiles 0 and 8 touch
    the window edge and take a constant triangle mask; ONE grouped exp
    activation covers all 9 score tiles [128, 1152].
  * Sequence-edge (first/last core chunks): K/V and RoPE tables are
    zero-padded, so out-of-sequence keys contribute exp(0)=1 to the
    softmax denominator; a host-computed count is subtracted from the
    denominator via a tiny extra PSUM-accumulate matmul.
  * o_proj: bf16 aoT x bf16 Wo (1 cyc/row).

Layouts: projections contract over hidden with hidden on partitions; Q/K
land as [head_dim, tokens] so scores^T and PV need no transposes; RMSNorm
partition reductions via ones-matmuls; softmax denominators via
ones-matmuls accumulated alongside PV; normalization applied post-PV via
a PE broadcast of reciprocal denominators.  Head dims are host-interleaved
[0,64,1,65,...] so RoPE's rotate_half is a single DVE stream-shuffle.
"""

import numpy as np


def _ensure_path():
    try:
        import concourse  # noqa: F401
    except ImportError:
        import sys
        for p in ("/opt/trn_rl_repo", "/root/.axon_site/_ro/trn_rl_repo"):
            if p not in sys.path:
                sys.path.insert(0, p)


H, KV, D = 16, 4, 128
GQ = H // KV            # 4 query heads per kv head
WIN = 512
EPS = 1e-6
B, L, HID = 2, 2048, 2048
OWN = 512               # tokens owned per core
HALO = 1536             # key/value token window per core (own +- 512)
NKT = HALO // 128       # 12 key tiles of 128
NHK = HID // 128        # 16 contraction tiles over hidden
NHP = NHK // 2          # 8 hidden-pair tiles (256-deep fp8 DoubleRow)
NQT = OWN // 128        # 4 diagonal query tiles of 128
NKD = 9                 # key tiles per 128-query window (1024+128)/128
N_CORES = 8
WSCALE = 64.0           # host-side fp8 weight scale
NEG = -1.0e30

_CACHE = {}


def _build():
    _ensure_path()
    import concourse.mybir as mybir
    import concourse.tile as tile
    from concourse import bacc
    from contextlib import ExitStack

    F32 = mybir.dt.float32
    F32R = mybir.dt.float32r
    F8 = mybir.dt.float8e4
    BF = mybir.dt.bfloat16
    DR = mybir.MatmulPerfMode.DoubleRow
    ACTF = mybir.ActivationFunctionType

    nc = bacc.Bacc("TRN2", target_bir_lowering=False, debug=False,
                   num_devices=N_CORES)

    x8T = nc.dram_tensor("x8T", [HID, HALO], F8, kind="ExternalInput").ap()
    xrT = nc.dram_tensor("xrT", [HID, HALO], F8, kind="ExternalInput").ap()
    WqT = nc.dram_tensor("WqT", [HID, H * D], F8, kind="ExternalInput").ap()
    WqrT = nc.dram_tensor("WqrT", [HID, H * D], F8, kind="ExternalInput").ap()
    WkT = nc.dram_tensor("WkT", [HID, KV * D], F8, kind="ExternalInput").ap()
    WkrT = nc.dram_tensor("WkrT", [HID, KV * D], F8,
                          kind="ExternalInput").ap()
    WvT = nc.dram_tensor("WvT", [HID, KV * D], F8, kind="ExternalInput").ap()
    WvrT = nc.dram_tensor("WvrT", [HID, KV * D], F8,
                          kind="ExternalInput").ap()
    WoT = nc.dram_tensor("WoT", [H * D, HID], BF, kind="ExternalInput").ap()
    # RoPE tables, transposed to [D, tokens], norm-weights (and for q the
    # 1/sqrt(D) score scale) folded in; s-table has rotate_half sign/roll.
    cqT = nc.dram_tensor("cqT", [D, OWN], F32, kind="ExternalInput").ap()
    sqT = nc.dram_tensor("sqT", [D, OWN], F32, kind="ExternalInput").ap()
    ckT = nc.dram_tensor("ckT", [D, HALO], F32, kind="ExternalInput").ap()
    skT = nc.dram_tensor("skT", [D, HALO], F32, kind="ExternalInput").ap()
    # [128, 2, 128] window-edge triangle masks (slot 0 -> kt0, 1 -> kt8)
    band = nc.dram_tensor("band", [128, 2, 128], F32,
                          kind="ExternalInput").ap()
    # negated out-of-sequence key counts per (q-tile, query)
    dcorr = nc.dram_tensor("dcorr", [1, NQT * 128], BF,
                           kind="ExternalInput").ap()
    out = nc.dram_tensor("out", [OWN, HID], F32, kind="ExternalOutput").ap()

    SWAP_MASK = [p ^ 1 for p in range(32)]

    with tile.TileContext(nc) as tc, ExitStack() as top:
        # ---- persistent SBUF ----
        keep = top.enter_context(tc.tile_pool(name="keep", bufs=1))
        v_sb = keep.tile([128, NKT, KV * D], BF)        # [tok128, ktile, vf]
        kT_sb = keep.tile([128, KV, HALO], BF)          # [d, kv, tok]
        qT_sb = keep.tile([128, H, OWN], BF)            # [d, h, tok]
        ones32 = keep.tile([128, 1], F32)
        nc.vector.memset(ones32, 1.0)
        ones_sb = keep.tile([128, 1], F32R)
        nc.vector.tensor_copy(ones_sb, ones32)
        ones_bf = keep.tile([128, 1], BF)
        nc.vector.tensor_copy(ones_bf, ones32)
        one1_bf = keep.tile([1, 1], BF)
        nc.vector.memset(one1_bf, 1.0)
        ones132 = keep.tile([1, 128], F32)
        nc.vector.memset(ones132, 1.0)
        ones1_sb = keep.tile([1, 128], F32R)
        nc.vector.tensor_copy(ones1_sb, ones132)
        band_sb = keep.tile([128, 2, 128], F32)
        nc.gpsimd.dma_start(out=band_sb, in_=band)
        dcorr_sb = keep.tile([1, NQT * 128], BF)
        nc.gpsimd.dma_start(out=dcorr_sb, in_=dcorr)

        # alternating resident zones: a phase's tensors prefetch while the
        # *other* zone's previous-phase readers drain
        zoneA = top.enter_context(tc.tile_pool(name="zoneA", bufs=1))
        zoneB = top.enter_context(tc.tile_pool(name="zoneB", bufs=1))
        xs = top.enter_context(tc.tile_pool(name="xs", bufs=18))
        ws = top.enter_context(tc.tile_pool(name="ws", bufs=18))

        def load_paired(dst, src, n):
            # [256r, C] DRAM rows -> [128, 2, C] fp8 tiles (k = 2p+s packing;
            # any bijection works since x and W tiles load identically)
            for k in range(n):
                nc.gpsimd.dma_start(out=dst[:, k],
                                    in_=src[k * 256:(k + 1) * 256, :])

        def load_x_terms(ch_lo, width):
            """Stream both x hi/lo fp8 tiles for a token slice."""
            tiles = []
            for src in (x8T, xrT):
                cur = []
                for k in range(NHP):
                    xt = xs.tile([128, 2, width], F8, tag="xt")
                    nc.sync.dma_start(
                        out=xt, in_=src[k * 256:(k + 1) * 256,
                                        ch_lo:ch_lo + width])
                    cur.append(xt)
                tiles.append(cur)
            return tiles

        # ========== V projection (3-term fp8 DR: x8W8 + xrW8 + x8Wr) =====
        wv_sb = zoneA.tile([128, NHP, 2, KV * D], F8, tag="wv", name="wv_sb")
        wvr_sb = zoneA.tile([128, NHP, 2, KV * D], F8, tag="wvr",
                            name="wvr_sb")
        load_paired(wv_sb, WvT, NHP)
        load_paired(wvr_sb, WvrT, NHP)
        with ExitStack() as ph:
            ps = ph.enter_context(tc.tile_pool(name="vps", bufs=8,
                                               space="PSUM"))
            for ch in range(3):
                pv = [ps.tile([128, KV * D], F32, tag="pv", name=f"pv{t}")
                      for t in range(4)]
                x8s, xrs = load_x_terms(ch * 512, 512)
                terms = [(x8s, wv_sb), (xrs, wv_sb), (x8s, wvr_sb)]
                for fh in range(2):
                    for t, (xts, wsb) in enumerate(terms):
                        for k in range(NHP):
                            for tt in range(4):
                                nc.tensor.matmul(
                                    pv[tt][:, fh * 256:(fh + 1) * 256],
                                    xts[k][:, :, tt * 128:(tt + 1) * 128],
                                    wsb[:, k, :, fh * 256:(fh + 1) * 256],
                                    start=(t == 0 and k == 0),
                                    stop=(t == 2 and k == NHP - 1),
                                    perf_mode=DR)
                for tt in range(4):
                    nc.scalar.copy(out=v_sb[:, ch * 4 + tt, :], in_=pv[tt])

        # ============ K / Q projection + RMSNorm + RoPE ============
        def norm_rope(p_feat, cT, sT, r_dst, n_tok, psn, scratch):
            """p_feat: psum [128 d, n_tok] raw head; writes r_dst (bf16)."""
            raw = scratch.tile([128, n_tok], F32, tag="raw")
            nc.scalar.copy(out=raw, in_=p_feat)
            sq = scratch.tile([128, n_tok], F32R, tag="sq")
            nc.vector.tensor_mul(out=sq, in0=raw, in1=raw)
            pss = psn.tile([1, n_tok], F32, tag="ss")
            nc.tensor.matmul(pss, ones_sb, sq, start=True, stop=True)
            ms = scratch.tile([1, n_tok], F32, tag="ms")
            nc.vector.tensor_scalar(out=ms, in0=pss, scalar1=1.0 / D,
                                    scalar2=EPS * WSCALE * WSCALE,
                                    op0=mybir.AluOpType.mult,
                                    op1=mybir.AluOpType.add)
            nc.vector.reciprocal(ms, ms)
            rs = scratch.tile([1, n_tok], F32R, tag="rs")
            nc.scalar.activation(out=rs, in_=ms, func=ACTF.Sqrt)
            prb = psn.tile([128, n_tok], F32, tag="rb")
            nc.tensor.matmul(prb, ones1_sb, rs, start=True, stop=True)
            swp = scratch.tile([128, n_tok], F32, tag="swp")
            nc.vector.stream_shuffle(out=swp, in_=raw, mask=SWAP_MASK)
            t1 = scratch.tile([128, n_tok], F32, tag="t1")
            nc.gpsimd.tensor_mul(out=t1, in0=raw, in1=cT)
            t2 = scratch.tile([128, n_tok], F32, tag="t2")
            nc.gpsimd.tensor_mul(out=t2, in0=swp, in1=sT)
            nc.gpsimd.tensor_add(out=t1, in0=t1, in1=t2)
            nc.vector.tensor_mul(out=r_dst, in0=t1, in1=prb)

        wk_sb = zoneB.tile([128, NHP, 2, KV * D], F8, tag="wk", name="wk_sb")
        wkr_sb = zoneB.tile([128, NHP, 2, KV * D], F8, tag="wkr",
                            name="wkr_sb")
        load_paired(wk_sb, WkT, NHP)
        load_paired(wkr_sb, WkrT, NHP)
        ck_sb = zoneB.tile([128, HALO], F32, tag="tc", name="ck_sb")
        sk_sb = zoneB.tile([128, HALO], F32, tag="ts", name="sk_sb")
        nc.gpsimd.dma_start(out=ck_sb, in_=ckT)
        nc.gpsimd.dma_start(out=sk_sb, in_=skT)

        with ExitStack() as ph:
            psp = ph.enter_context(tc.tile_pool(name="psp", bufs=6,
                                                space="PSUM"))
            psn = ph.enter_context(tc.tile_pool(name="psn", bufs=1,
                                                space="PSUM"))
            scratch = ph.enter_context(tc.tile_pool(name="scratch", bufs=2))
            for ch in range(3):
                pk = [psp.tile([128, 512], F32, tag="p", name=f"pk{t}")
                      for t in range(KV)]
                x8s, xrs = load_x_terms(ch * 512, 512)
                terms = [(x8s, wk_sb), (xrs, wk_sb), (x8s, wkr_sb)]
                for th in range(2):
                    for t, (xts, wsb) in enumerate(terms):
                        for k in range(NHP):
                            for h in range(KV):
                                nc.tensor.matmul(
                                    pk[h][:, th * 256:(th + 1) * 256],
                                    wsb[:, k, :, h * 128:(h + 1) * 128],
                                    xts[k][:, :, th * 256:(th + 1) * 256],
                                    start=(t == 0 and k == 0),
                                    stop=(t == 2 and k == NHP - 1),
                                    perf_mode=DR)
                sl = slice(ch * 512, (ch + 1) * 512)
                for h in range(KV):
                    norm_rope(pk[h], ck_sb[:, sl], sk_sb[:, sl],
                              kT_sb[:, h, sl], 512, psn, scratch)

            # ---- Q: own-x terms resident in zone A (Wv readers done) ----
            xo8_sb = zoneA.tile([128, NHP, 2, OWN], F8, tag="xo8",
                                name="xo8_sb")
            xor_sb = zoneA.tile([128, NHP, 2, OWN], F8, tag="xor",
                                name="xor_sb")
            load_paired(xo8_sb, x8T[:, 512:1024], NHP)
            load_paired(xor_sb, xrT[:, 512:1024], NHP)
            cq_sb = zoneA.tile([128, OWN], F32, tag="tc", name="cq_sb")
            sq_sb = zoneA.tile([128, OWN], F32, tag="ts", name="sq_sb")
            nc.gpsimd.dma_start(out=cq_sb, in_=cqT)
            nc.gpsimd.dma_start(out=sq_sb, in_=sqT)
            for qf in range(4):
                pq = [psp.tile([128, OWN], F32, tag="p", name=f"pq{t}")
                      for t in range(4)]
                wqs, wqrs = [], []
                for src, dst in ((WqT, wqs), (WqrT, wqrs)):
                    for k in range(NHP):
                        wq = ws.tile([128, 2, 512], F8, tag="w")
                        nc.sync.dma_start(
                            out=wq, in_=src[k * 256:(k + 1) * 256,
                                            qf * 512:(qf + 1) * 512])
                        dst.append(wq)
                terms = [(xo8_sb, wqs), (xor_sb, wqs), (xo8_sb, wqrs)]
                for th in range(2):
                    for t, (xo, wql) in enumerate(terms):
                        for k in range(NHP):
                            for j in range(4):
                                nc.tensor.matmul(
                                    pq[j][:, th * 256:(th + 1) * 256],
                                    wql[k][:, :, j * 128:(j + 1) * 128],
                                    xo[:, k, :, th * 256:(th + 1) * 256],
                                    start=(t == 0 and k == 0),
                                    stop=(t == 2 and k == NHP - 1),
                                    perf_mode=DR)
                for j in range(4):
                    norm_rope(pq[j], cq_sb, sq_sb, qT_sb[:, qf * 4 + j, :],
                              OWN, psn, scratch)

        # ================= attention (bf16, diagonal 128-q tiles) ========
        aoT_sb = zoneA.tile([128, H, OWN], BF, tag="ao", name="aoT_sb")
        with ExitStack() as ph:
            # PSUM banks: scores+pb ring 2x3 + po 1 + pd 1 = 8
            pssc = ph.enter_context(tc.tile_pool(name="aps", bufs=2,
                                                 space="PSUM"))
            pso = ph.enter_context(tc.tile_pool(name="apo", bufs=1,
                                                space="PSUM"))
            psd = ph.enter_context(tc.tile_pool(name="apd", bufs=1,
                                                space="PSUM"))
            es = ph.enter_context(tc.tile_pool(name="aes", bufs=3))
            sc = ph.enter_context(tc.tile_pool(name="asc", bufs=2))
            # window-edge tiles first so their mask adds overlap the rest
            JORDER = [0, NKD - 1] + list(range(1, NKD - 1))
            for h in range(H):
                kv = h // GQ
                po_h = pso.tile([128, OWN], F32, tag="po")
                pd_h = psd.tile([1, OWN], F32, tag="pd")
                for qt in range(NQT):
                    qsl = slice(qt * 128, (qt + 1) * 128)
                    ps_s = pssc.tile([128, NKD * 128], F32, tag="s")
                    for j in JORDER:
                        nc.tensor.matmul(
                            ps_s[:, j * 128:(j + 1) * 128],
                            kT_sb[:, kv, (qt + j) * 128:(qt + j + 1) * 128],
                            qT_sb[:, h, qsl], start=True, stop=True)
                        if j == 0:
                            nc.vector.tensor_add(
                                out=ps_s[:, :128], in0=ps_s[:, :128],
                                in1=band_sb[:, 0, :])
                        elif j == NKD - 1:
                            nc.vector.tensor_add(
                                out=ps_s[:, (NKD - 1) * 128:],
                                in0=ps_s[:, (NKD - 1) * 128:],
                                in1=band_sb[:, 1, :])
                    e = es.tile([128, NKD * 128], BF, tag="e")
                    nc.scalar.activation(out=e, in_=ps_s, func=ACTF.Exp)
                    for j in range(NKD):
                        esl = e[:, j * 128:(j + 1) * 128]
                        nc.tensor.matmul(po_h[:, qsl],
                                         v_sb[:, qt + j,
                                              kv * 128:(kv + 1) * 128],
                                         esl, start=(j == 0),
                                         stop=(j == NKD - 1),
                                         skip_group_check=True)
                        nc.tensor.matmul(pd_h[:, qsl], ones_bf, esl,
                                         start=(j == 0), stop=False,
                                         skip_group_check=True)
                    nc.tensor.matmul(pd_h[:, qsl], one1_bf, dcorr_sb[:, qsl],
                                     start=False, stop=True,
                                     skip_group_check=True)
                dr32 = sc.tile([1, OWN], F32, tag="dr32")
                nc.vector.reciprocal(dr32, pd_h)
                drr = sc.tile([1, OWN], F32R, tag="drr")
                nc.vector.tensor_copy(drr, dr32)
                pbt = pssc.tile([128, NKD * 128], F32, tag="s", name="pbt")
                pb = pbt[:, :OWN]
                nc.tensor.matmul(pb, ones1_sb, drr, start=True, stop=True)
                bf = sc.tile([128, OWN], F32, tag="bf")
                nc.vector.tensor_copy(bf, pb)
                nc.vector.tensor_mul(out=aoT_sb[:, h, :], in0=po_h, in1=bf)

        # ================= output projection (bf16) =================
        with ExitStack() as ph:
            psy = ph.enter_context(tc.tile_pool(name="ops", bufs=8,
                                                space="PSUM"))
            ys = ph.enter_context(tc.tile_pool(name="oy", bufs=4))
            for hc in range(4):
                py = [psy.tile([128, 512], F32, tag="py", name=f"py{t}")
                      for t in range(4)]
                for h in range(H):
                    wo = ws.tile([128, 512], BF, tag="w")
                    eng = nc.sync if h % 2 == 0 else nc.scalar
                    eng.dma_start(
                        out=wo, in_=WoT[h * 128:(h + 1) * 128,
                                        hc * 512:(hc + 1) * 512])
                    for tt in range(4):
                        nc.tensor.matmul(py[tt],
                                         aoT_sb[:, h, tt * 128:(tt + 1) * 128],
                                         wo, start=(h == 0), stop=(h == H - 1))
                for tt in range(4):
                    y = ys.tile([128, 512], F32, tag="y")
                    nc.vector.tensor_scalar(out=y, in0=py[tt],
                                            scalar1=1.0 / WSCALE, scalar2=0.0,
                                            op0=mybir.AluOpType.mult,
                                            op1=mybir.AluOpType.add)
                    nc.sync.dma_start(
                        out=out[tt * 128:(tt + 1) * 128,
                                hc * 512:(hc + 1) * 512], in_=y)

    nc.compile()
    return nc


def _host_prep(x, cos, sin, Wq, Wk, Wv, Wo, q_norm_w, k_norm_w):
    """Build the 8 per-core input dicts."""
    import ml_dtypes
    f8 = ml_dtypes.float8_e4m3
    bf16 = ml_dtypes.bfloat16
    scale = 1.0 / np.sqrt(D)
    # interleave head dims [0,64,1,65,...]: rotate_half partners end up on
    # adjacent partitions so the kernel swaps them with one stream-shuffle
    perm = np.empty(D, np.int64)
    perm[0::2] = np.arange(64)
    perm[1::2] = 64 + np.arange(64)

    def rope_tables(cos_r, sin_r, w, extra):
        # fold norm weight (and any extra scale); sign/roll for rotate_half
        c = (cos_r * w[None, :] * extra).astype(np.float32)
        w_rot = np.roll(w, -64)
        s = (sin_r * w_rot[None, :] * extra).astype(np.float32)
        s[:, :64] *= -1.0
        return (np.ascontiguousarray(c.T[perm]),
                np.ascontiguousarray(s.T[perm]))

    idx_q = (np.arange(H)[:, None] * D + perm[None, :]).ravel()
    idx_k = (np.arange(KV)[:, None] * D + perm[None, :]).ravel()

    def split8(w):
        hi = w.astype(f8)
        lo = (w - hi.astype(np.float32)).astype(f8)
        return hi, lo

    WqT, WqrT = split8(np.ascontiguousarray(Wq.T[:, idx_q] * WSCALE))
    WkT, WkrT = split8(np.ascontiguousarray(Wk.T[:, idx_k] * WSCALE))
    WvT, WvrT = split8(np.ascontiguousarray(Wv.T * WSCALE))
    WoT = np.ascontiguousarray(Wo.T).astype(bf16)

    # window-edge triangle masks for key tiles 0 and 8 of each 128-q window
    jj = np.arange(128)[:, None]
    ii = np.arange(128)[None, :]
    band = np.zeros((128, 2, 128), np.float32)
    band[:, 0, :] = np.where(jj < ii, NEG, 0.0)   # kt0: key - query < -512
    band[:, 1, :] = np.where(jj > ii, NEG, 0.0)   # kt8: key - query > +512

    in_maps = []
    for c in range(N_CORES):
        b, ch = divmod(c, 4)
        start = ch * OWN
        lo, hi = start - WIN, start + OWN + WIN
        vlo, vhi = max(lo, 0), min(hi, L)
        xh = np.zeros((HALO, HID), np.float32)
        xh[vlo - lo:vhi - lo] = x[b, vlo:vhi]
        ch_cos = np.zeros((HALO, D), np.float32)
        ch_sin = np.zeros((HALO, D), np.float32)
        ch_cos[vlo - lo:vhi - lo] = cos[vlo:vhi]
        ch_sin[vlo - lo:vhi - lo] = sin[vlo:vhi]
        ckT, skT = rope_tables(ch_cos, ch_sin, k_norm_w, 1.0)
        cqT, sqT = rope_tables(cos[start:start + OWN], sin[start:start + OWN],
                               q_norm_w, scale)
        # hi/lo fp8 split of the (transposed) halo activations
        xhT = np.ascontiguousarray(xh.T)
        x8 = xhT.astype(f8)
        xr = (xhT - x8.astype(np.float32)).astype(f8)
        # negated count of out-of-sequence keys reaching the softmax
        # denominator as exp(0)=1 (zero-padded K => score 0, unless the
        # window-edge triangle already masks that (j, i) entry)
        dcorr = np.zeros((NQT, 128), np.float32)
        for qt in range(NQT):
            q0g = start + qt * 128
            kk = q0g - WIN + np.arange(NKD * 128)
            invalid = (kk < 0) | (kk >= L)          # [1152]
            inv = invalid.reshape(NKD, 128)         # [kt, j]
            for i in range(128):
                cnt = inv.sum()
                cnt -= inv[0, :][jj[:, 0] < i].sum()   # kt0 triangle masked
                cnt -= inv[8, :][jj[:, 0] > i].sum()   # kt8 triangle masked
                dcorr[qt, i] = -float(cnt)
        in_maps.append({
            "x8T": x8, "xrT": xr,
            "WqT": WqT, "WqrT": WqrT, "WkT": WkT, "WkrT": WkrT,
            "WvT": WvT, "WvrT": WvrT, "WoT": WoT,
            "cqT": cqT, "sqT": sqT, "ckT": ckT, "skT": skT,
            "band": band, "dcorr": dcorr.reshape(1, NQT * 128).astype(bf16),
        })
    return in_maps


def kernel(**inputs):
    _ensure_path()
    from concourse import bass_utils

    if "nc" not in _CACHE:
        _CACHE["nc"] = _build()
    nc = _CACHE["nc"]

    in_maps = _host_prep(
        np.asarray(inputs["x"]), np.asarray(inputs["cos"]),
        np.asarray(inputs["sin"]), np.asarray(inputs["Wq"]),
        np.asarray(inputs["Wk"]), np.asarray(inputs["Wv"]),
        np.asarray(inputs["Wo"]), np.asarray(inputs["q_norm_w"]),
        np.asarray(inputs["k_norm_w"]))

    res = bass_utils.run_bass_kernel_spmd(nc, in_maps,
                                          core_ids=list(range(N_CORES)))
    out = np.empty((B, L, HID), np.float32)
    for c in range(N_CORES):
        b, ch = divmod(c, 4)
        out[b, ch * OWN:(ch + 1) * OWN] = res.results[c]["out"]
    return out


# revision 41
# speedup vs baseline: 1.1508x; 1.0916x over previous
"""Trainium2 Bass kernel: sliding-window GQA attention block.

Computation (matches the PyTorch/JAX reference):
    q,k,v = x @ {Wq,Wk,Wv}.T ; QK-RMSNorm ; RoPE ; GQA repeat(4x) ;
    softmax(q k^T / sqrt(D) + sliding-window bias(|i-j|<=512)) v ; @ Wo.T

Sharding (no collectives): 8 cores = 2 batches x 4 sequence chunks of 512
tokens.  Each core computes its 512 own tokens for ALL 16 heads, using a
512-token halo either side for K/V (halo K/V recomputed locally), then the
full o_proj rows for its tokens.  Outputs concatenate on host.

Precision/speed split:
  * Q/K/V projections: fp8(e4m3) DoubleRow matmuls (0.5 cyc/row, 256-deep
    contraction) with the activation split hi+lo into TWO fp8 terms
    (x = x8 + xr) accumulated in PSUM; weights single fp8 scaled by 64
    (RMSNorm makes Q/K scale-invariant; V's 64x rides through attention).
    Net cost 2 DR units = half of fp32r, with only the W-quantization
    error (~the fp8 step of W) remaining.
  * Attention: bf16 operands (kT/qT/v/e), full-rate 1 cyc/row at any
    moving size, which enables diagonal 128-query tiling: per (head,
    128-q tile) exactly 9 key tiles are live; only tiles 0 and 8 touch
    the window edge and take a constant triangle mask; ONE grouped exp
    activation covers all 9 score tiles [128, 1152].
  * Sequence-edge (first/last core chunks): K/V and RoPE tables are
    zero-padded, so out-of-sequence keys contribute exp(0)=1 to the
    softmax denominator; a host-computed count is subtracted from the
    denominator via a tiny extra PSUM-accumulate matmul.
  * o_proj: bf16 aoT x bf16 Wo (1 cyc/row).

Layouts: projections contract over hidden with hidden on partitions; Q/K
land as [head_dim, tokens] so scores^T and PV need no transposes; RMSNorm
partition reductions via ones-matmuls; softmax denominators via
ones-matmuls accumulated alongside PV; normalization applied post-PV via
a PE broadcast of reciprocal denominators.  Head dims are host-interleaved
[0,64,1,65,...] so RoPE's rotate_half is a single DVE stream-shuffle.
"""

import numpy as np


def _ensure_path():
    try:
        import concourse  # noqa: F401
    except ImportError:
        import sys
        for p in ("/opt/trn_rl_repo", "/root/.axon_site/_ro/trn_rl_repo"):
            if p not in sys.path:
                sys.path.insert(0, p)


H, KV, D = 16, 4, 128
GQ = H // KV            # 4 query heads per kv head
WIN = 512
EPS = 1e-6
B, L, HID = 2, 2048, 2048
OWN = 512               # tokens owned per core
HALO = 1536             # key/value token window per core (own +- 512)
NKT = HALO // 128       # 12 key tiles of 128
NHK = HID // 128        # 16 contraction tiles over hidden
NHP = NHK // 2          # 8 hidden-pair tiles (256-deep fp8 DoubleRow)
NQT = OWN // 128        # 4 diagonal query tiles of 128
NKD = 9                 # key tiles per 128-query window (1024+128)/128
N_CORES = 8
WSCALE = 64.0           # host-side fp8 weight scale
NEG = -1.0e30

_CACHE = {}


def _build():
    _ensure_path()
    import concourse.mybir as mybir
    import concourse.tile as tile
    from concourse import bacc
    from contextlib import ExitStack

    F32 = mybir.dt.float32
    F32R = mybir.dt.float32r
    F8 = mybir.dt.float8e4
    BF = mybir.dt.bfloat16
    DR = mybir.MatmulPerfMode.DoubleRow
    ACTF = mybir.ActivationFunctionType

    nc = bacc.Bacc("TRN2", target_bir_lowering=False, debug=False,
                   num_devices=N_CORES)

    x8T = nc.dram_tensor("x8T", [HID, HALO], F8, kind="ExternalInput").ap()
    xrT = nc.dram_tensor("xrT", [HID, HALO], F8, kind="ExternalInput").ap()
    WqT = nc.dram_tensor("WqT", [HID, H * D], F8, kind="ExternalInput").ap()
    WqrT = nc.dram_tensor("WqrT", [HID, H * D], F8, kind="ExternalInput").ap()
    WkT = nc.dram_tensor("WkT", [HID, KV * D], F8, kind="ExternalInput").ap()
    WkrT = nc.dram_tensor("WkrT", [HID, KV * D], F8,
                          kind="ExternalInput").ap()
    WvT = nc.dram_tensor("WvT", [HID, KV * D], F8, kind="ExternalInput").ap()
    WvrT = nc.dram_tensor("WvrT", [HID, KV * D], F8,
                          kind="ExternalInput").ap()
    WoT = nc.dram_tensor("WoT", [H * D, HID], F8, kind="ExternalInput").ap()
    WorT = nc.dram_tensor("WorT", [H * D, HID], F8,
                          kind="ExternalInput").ap()
    # RoPE tables, transposed to [D, tokens], norm-weights (and for q the
    # 1/sqrt(D) score scale) folded in; s-table has rotate_half sign/roll.
    cqT = nc.dram_tensor("cqT", [D, OWN], F32, kind="ExternalInput").ap()
    sqT = nc.dram_tensor("sqT", [D, OWN], F32, kind="ExternalInput").ap()
    ckT = nc.dram_tensor("ckT", [D, HALO], F32, kind="ExternalInput").ap()
    skT = nc.dram_tensor("skT", [D, HALO], F32, kind="ExternalInput").ap()
    # [128, 2, 128] window-edge triangle masks (slot 0 -> kt0, 1 -> kt8)
    band = nc.dram_tensor("band", [128, 2, 128], F32,
                          kind="ExternalInput").ap()
    # negated out-of-sequence key counts per (q-tile, query)
    dcorr = nc.dram_tensor("dcorr", [1, NQT * 128], BF,
                           kind="ExternalInput").ap()
    out = nc.dram_tensor("out", [OWN, HID], F32, kind="ExternalOutput").ap()

    SWAP_MASK = [p ^ 1 for p in range(32)]

    with tile.TileContext(nc) as tc, ExitStack() as top:
        # ---- persistent SBUF ----
        keep = top.enter_context(tc.tile_pool(name="keep", bufs=1))
        v_sb = keep.tile([128, NKT, KV * D], BF)        # [tok128, ktile, vf]
        kT_sb = keep.tile([128, KV, HALO], BF)          # [d, kv, tok]
        qT_sb = keep.tile([128, H, OWN], BF)            # [d, h, tok]
        ones32 = keep.tile([128, 1], F32)
        nc.vector.memset(ones32, 1.0)
        ones_sb = keep.tile([128, 1], F32R)
        nc.vector.tensor_copy(ones_sb, ones32)
        ones_bf = keep.tile([128, 1], BF)
        nc.vector.tensor_copy(ones_bf, ones32)
        one1_bf = keep.tile([1, 1], BF)
        nc.vector.memset(one1_bf, 1.0)
        ones132 = keep.tile([1, 128], F32)
        nc.vector.memset(ones132, 1.0)
        ones1_sb = keep.tile([1, 128], F32R)
        nc.vector.tensor_copy(ones1_sb, ones132)
        band_sb = keep.tile([128, 2, 128], F32)
        nc.gpsimd.dma_start(out=band_sb, in_=band)
        dcorr_sb = keep.tile([1, NQT * 128], BF)
        nc.gpsimd.dma_start(out=dcorr_sb, in_=dcorr)

        # alternating resident zones: a phase's tensors prefetch while the
        # *other* zone's previous-phase readers drain
        zoneA = top.enter_context(tc.tile_pool(name="zoneA", bufs=1))
        zoneB = top.enter_context(tc.tile_pool(name="zoneB", bufs=1))
        ws = top.enter_context(tc.tile_pool(name="ws", bufs=3))

        def load_paired(dst, src, n, eng=None):
            # [256r, C] DRAM rows -> [128, n, 2, C] fp8 in ONE DMA (k = 2p+s
            # packing; any bijection works since x and W load identically)
            (eng or nc.gpsimd).dma_start(out=dst, in_=src)

        # persistent hi/lo fp8 halo activations, one big DMA each (the DMA's
        # AP does the [2048r, 1536] -> [128, 8, 2, 1536] pair packing)
        x8c = keep.tile([128, NHP, 2, HALO], F8)
        xrc = keep.tile([128, NHP, 2, HALO], F8)
        nc.sync.dma_start(out=x8c, in_=x8T)
        nc.sync.dma_start(out=xrc, in_=xrT)

        # ========== V projection (3-term fp8 DR: x8W8 + xrW8 + x8Wr) =====
        wv_sb = zoneA.tile([128, NHP, 2, KV * D], F8, tag="wv", name="wv_sb")
        wvr_sb = zoneA.tile([128, NHP, 2, KV * D], F8, tag="wvr",
                            name="wvr_sb")
        load_paired(wv_sb, WvT, NHP)
        load_paired(wvr_sb, WvrT, NHP)
        with ExitStack() as ph:
            ps = ph.enter_context(tc.tile_pool(name="vps", bufs=8,
                                               space="PSUM"))
            for ch in range(3):
                pv = [ps.tile([128, KV * D], F32, tag="pv", name=f"pv{t}")
                      for t in range(4)]
                c0 = ch * 512
                terms = [(x8c, wv_sb), (xrc, wv_sb), (x8c, wvr_sb)]
                for fh in range(2):
                    for t, (xc, wsb) in enumerate(terms):
                        for k in range(NHP):
                            for tt in range(4):
                                nc.tensor.matmul(
                                    pv[tt][:, fh * 256:(fh + 1) * 256],
                                    xc[:, k, :,
                                       c0 + tt * 128:c0 + (tt + 1) * 128],
                                    wsb[:, k, :, fh * 256:(fh + 1) * 256],
                                    start=(t == 0 and k == 0),
                                    stop=(t == 2 and k == NHP - 1),
                                    perf_mode=DR)
                for tt in range(4):
                    nc.scalar.copy(out=v_sb[:, ch * 4 + tt, :], in_=pv[tt])

        # ============ K / Q projection + RMSNorm + RoPE ============
        def norm_rope(p_feat, cT, sT, r_dst, n_tok, psn, scratch):
            """p_feat: psum [128 d, n_tok] raw head; writes r_dst (bf16)."""
            raw = scratch.tile([128, n_tok], F32, tag="raw")
            nc.scalar.copy(out=raw, in_=p_feat)
            sq = scratch.tile([128, n_tok], F32R, tag="sq")
            nc.vector.tensor_mul(out=sq, in0=raw, in1=raw)
            pss = psn.tile([1, n_tok], F32, tag="ss")
            nc.tensor.matmul(pss, ones_sb, sq, start=True, stop=True)
            ms = scratch.tile([1, n_tok], F32, tag="ms")
            nc.vector.tensor_scalar(out=ms, in0=pss, scalar1=1.0 / D,
                                    scalar2=EPS * WSCALE * WSCALE,
                                    op0=mybir.AluOpType.mult,
                                    op1=mybir.AluOpType.add)
            nc.vector.reciprocal(ms, ms)
            rs = scratch.tile([1, n_tok], F32R, tag="rs")
            nc.scalar.activation(out=rs, in_=ms, func=ACTF.Sqrt)
            prb = psn.tile([128, n_tok], F32, tag="rb")
            nc.tensor.matmul(prb, ones1_sb, rs, start=True, stop=True)
            swp = scratch.tile([128, n_tok], F32, tag="swp")
            nc.vector.stream_shuffle(out=swp, in_=raw, mask=SWAP_MASK)
            t1 = scratch.tile([128, n_tok], F32, tag="t1")
            nc.gpsimd.tensor_mul(out=t1, in0=raw, in1=cT)
            t2 = scratch.tile([128, n_tok], F32, tag="t2")
            nc.gpsimd.tensor_mul(out=t2, in0=swp, in1=sT)
            nc.gpsimd.tensor_add(out=t1, in0=t1, in1=t2)
            nc.vector.tensor_mul(out=r_dst, in0=t1, in1=prb)

        wk_sb = zoneB.tile([128, NHP, 2, KV * D], F8, tag="wk", name="wk_sb")
        wkr_sb = zoneB.tile([128, NHP, 2, KV * D], F8, tag="wkr",
                            name="wkr_sb")
        load_paired(wk_sb, WkT, NHP)
        load_paired(wkr_sb, WkrT, NHP)
        ck_sb = zoneB.tile([128, HALO], F32, tag="tc", name="ck_sb")
        sk_sb = zoneB.tile([128, HALO], F32, tag="ts", name="sk_sb")
        nc.gpsimd.dma_start(out=ck_sb, in_=ckT)
        nc.gpsimd.dma_start(out=sk_sb, in_=skT)

        with ExitStack() as ph:
            psp = ph.enter_context(tc.tile_pool(name="psp", bufs=6,
                                                space="PSUM"))
            psn = ph.enter_context(tc.tile_pool(name="psn", bufs=1,
                                                space="PSUM"))
            scratch = ph.enter_context(tc.tile_pool(name="scratch", bufs=2))
            for ch in range(3):
                pk = [psp.tile([128, 512], F32, tag="p", name=f"pk{t}")
                      for t in range(KV)]
                c0 = ch * 512
                terms = [(x8c, wk_sb), (xrc, wk_sb), (x8c, wkr_sb)]
                for th in range(2):
                    for t, (xc, wsb) in enumerate(terms):
                        for k in range(NHP):
                            for h in range(KV):
                                nc.tensor.matmul(
                                    pk[h][:, th * 256:(th + 1) * 256],
                                    wsb[:, k, :, h * 128:(h + 1) * 128],
                                    xc[:, k, :,
                                       c0 + th * 256:c0 + (th + 1) * 256],
                                    start=(t == 0 and k == 0),
                                    stop=(t == 2 and k == NHP - 1),
                                    perf_mode=DR)
                sl = slice(ch * 512, (ch + 1) * 512)
                for h in range(KV):
                    norm_rope(pk[h], ck_sb[:, sl], sk_sb[:, sl],
                              kT_sb[:, h, sl], 512, psn, scratch)

            # ---- Q: own-token slices of the persistent x terms ----
            cq_sb = zoneA.tile([128, OWN], F32, tag="tc", name="cq_sb")
            sq_sb = zoneA.tile([128, OWN], F32, tag="ts", name="sq_sb")
            nc.gpsimd.dma_start(out=cq_sb, in_=cqT)
            nc.gpsimd.dma_start(out=sq_sb, in_=sqT)
            for qf in range(4):
                pq = [psp.tile([128, OWN], F32, tag="p", name=f"pq{t}")
                      for t in range(4)]
                wqs, wqrs = [], []
                for src, dst in ((WqT, wqs), (WqrT, wqrs)):
                    wq = ws.tile([128, NHP, 2, 512], F8, tag="w")
                    nc.sync.dma_start(
                        out=wq, in_=src[:, qf * 512:(qf + 1) * 512])
                    dst.append(wq)
                terms = [(x8c, wqs[0]), (xrc, wqs[0]), (x8c, wqrs[0])]
                for th in range(2):
                    for t, (xc, wql) in enumerate(terms):
                        for k in range(NHP):
                            for j in range(4):
                                nc.tensor.matmul(
                                    pq[j][:, th * 256:(th + 1) * 256],
                                    wql[:, k, :, j * 128:(j + 1) * 128],
                                    xc[:, k, :,
                                       512 + th * 256:512 + (th + 1) * 256],
                                    start=(t == 0 and k == 0),
                                    stop=(t == 2 and k == NHP - 1),
                                    perf_mode=DR)
                for j in range(4):
                    norm_rope(pq[j], cq_sb, sq_sb, qT_sb[:, qf * 4 + j, :],
                              OWN, psn, scratch)

        # ================= attention (bf16, diagonal 128-q tiles) ========
        aoT8_sb = zoneA.tile([128, H, OWN], F8, tag="ao8", name="aoT8_sb")
        aoTr_sb = zoneA.tile([128, H, OWN], F8, tag="aor", name="aoTr_sb")
        with ExitStack() as ph:
            # PSUM banks: scores-A(+pb) ring 2x2 + scores-B ring 2x1
            # + po 1 + pd 1 = 8.  kt8 is scored and exp'd FIRST so PV can
            # begin while the big exp-A drains, keeping the PE fed.
            pssc = ph.enter_context(tc.tile_pool(name="aps", bufs=2,
                                                 space="PSUM"))
            pssb = ph.enter_context(tc.tile_pool(name="apsb", bufs=2,
                                                 space="PSUM"))
            pso = ph.enter_context(tc.tile_pool(name="apo", bufs=1,
                                                space="PSUM"))
            psd = ph.enter_context(tc.tile_pool(name="apd", bufs=1,
                                                space="PSUM"))
            es = ph.enter_context(tc.tile_pool(name="aes", bufs=3))
            eb = ph.enter_context(tc.tile_pool(name="aeb", bufs=3))
            sc = ph.enter_context(tc.tile_pool(name="asc", bufs=2))
            for h in range(H):
                kv = h // GQ
                po_h = pso.tile([128, OWN], F32, tag="po")
                pd_h = psd.tile([1, OWN], F32, tag="pd")
                for qt in range(NQT):
                    qsl = slice(qt * 128, (qt + 1) * 128)
                    ps_b = pssb.tile([128, 128], F32, tag="sb")
                    nc.tensor.matmul(
                        ps_b,
                        kT_sb[:, kv, (qt + 8) * 128:(qt + 9) * 128],
                        qT_sb[:, h, qsl], start=True, stop=True)
                    nc.vector.tensor_add(out=ps_b, in0=ps_b,
                                         in1=band_sb[:, 1, :])
                    e_b = eb.tile([128, 128], BF, tag="eb")
                    nc.scalar.activation(out=e_b, in_=ps_b, func=ACTF.Exp)
                    ps_s = pssc.tile([128, 8 * 128], F32, tag="s")
                    for j in range(8):
                        nc.tensor.matmul(
                            ps_s[:, j * 128:(j + 1) * 128],
                            kT_sb[:, kv, (qt + j) * 128:(qt + j + 1) * 128],
                            qT_sb[:, h, qsl], start=True, stop=True)
                        if j == 0:
                            nc.vector.tensor_add(
                                out=ps_s[:, :128], in0=ps_s[:, :128],
                                in1=band_sb[:, 0, :])
                    e = es.tile([128, 8 * 128], BF, tag="e")
                    nc.scalar.activation(out=e, in_=ps_s, func=ACTF.Exp)
                    nc.tensor.matmul(po_h[:, qsl],
                                     v_sb[:, qt + 8, kv * 128:(kv + 1) * 128],
                                     e_b, start=True, stop=False,
                                     skip_group_check=True)
                    nc.tensor.matmul(pd_h[:, qsl], ones_bf, e_b,
                                     start=True, stop=False,
                                     skip_group_check=True)
                    for j in range(8):
                        esl = e[:, j * 128:(j + 1) * 128]
                        nc.tensor.matmul(po_h[:, qsl],
                                         v_sb[:, qt + j,
                                              kv * 128:(kv + 1) * 128],
                                         esl, start=False, stop=(j == 7),
                                         skip_group_check=True)
                        nc.tensor.matmul(pd_h[:, qsl], ones_bf, esl,
                                         start=False, stop=False,
                                         skip_group_check=True)
                    nc.tensor.matmul(pd_h[:, qsl], one1_bf, dcorr_sb[:, qsl],
                                     start=False, stop=True,
                                     skip_group_check=True)
                dr32 = sc.tile([1, OWN], F32, tag="dr32")
                nc.vector.reciprocal(dr32, pd_h)
                drr = sc.tile([1, OWN], F32R, tag="drr")
                nc.vector.tensor_copy(drr, dr32)
                pbt = pssc.tile([128, 8 * 128], F32, tag="s", name="pbt")
                pb = pbt[:, :OWN]
                nc.tensor.matmul(pb, ones1_sb, drr, start=True, stop=True)
                bf = sc.tile([128, OWN], F32, tag="bf")
                nc.vector.tensor_copy(bf, pb)
                tf = sc.tile([128, OWN], F32, tag="tf")
                nc.vector.tensor_mul(out=tf, in0=po_h, in1=bf)
                nc.vector.tensor_copy(aoT8_sb[:, h, :], tf)
                nc.vector.tensor_tensor(out=aoTr_sb[:, h, :], in0=tf,
                                        in1=aoT8_sb[:, h, :],
                                        op=mybir.AluOpType.subtract)

        # ============ output projection (3-term fp8 DR) ============
        # Wo DRAM rows are host-permuted so a single paired DMA lands
        # [128, 8hp, 2hs, cols] matching aoT's (d, head) feature layout.
        with ExitStack() as ph:
            psy = ph.enter_context(tc.tile_pool(name="ops", bufs=8,
                                                space="PSUM"))
            ys = ph.enter_context(tc.tile_pool(name="oy", bufs=4))
            for hc in range(4):
                py = [psy.tile([128, 512], F32, tag="py", name=f"py{t}")
                      for t in range(4)]
                wo8 = ws.tile([128, H // 2, 2, 512], F8, tag="w")
                wor = ws.tile([128, H // 2, 2, 512], F8, tag="w")
                nc.sync.dma_start(out=wo8,
                                  in_=WoT[:, hc * 512:(hc + 1) * 512])
                nc.scalar.dma_start(out=wor,
                                    in_=WorT[:, hc * 512:(hc + 1) * 512])
                terms = [(aoT8_sb, wo8), (aoTr_sb, wo8), (aoT8_sb, wor)]
                for wh in range(2):
                    for t, (ao, wo) in enumerate(terms):
                        for hp in range(H // 2):
                            for tt in range(4):
                                nc.tensor.matmul(
                                    py[tt][:, wh * 256:(wh + 1) * 256],
                                    ao[:, 2 * hp:2 * hp + 2,
                                       tt * 128:(tt + 1) * 128],
                                    wo[:, hp, :, wh * 256:(wh + 1) * 256],
                                    start=(t == 0 and hp == 0),
                                    stop=(t == 2 and hp == H // 2 - 1),
                                    perf_mode=DR)
                for tt in range(4):
                    y = ys.tile([128, 512], F32, tag="y")
                    nc.vector.tensor_scalar(out=y, in0=py[tt],
                                            scalar1=1.0 / (WSCALE * WSCALE),
                                            scalar2=0.0,
                                            op0=mybir.AluOpType.mult,
                                            op1=mybir.AluOpType.add)
                    nc.sync.dma_start(
                        out=out[tt * 128:(tt + 1) * 128,
                                hc * 512:(hc + 1) * 512], in_=y)

    nc.compile()
    return nc


def _host_prep(x, cos, sin, Wq, Wk, Wv, Wo, q_norm_w, k_norm_w):
    """Build the 8 per-core input dicts."""
    import ml_dtypes
    f8 = ml_dtypes.float8_e4m3
    bf16 = ml_dtypes.bfloat16
    scale = 1.0 / np.sqrt(D)
    # interleave head dims [0,64,1,65,...]: rotate_half partners end up on
    # adjacent partitions so the kernel swaps them with one stream-shuffle
    perm = np.empty(D, np.int64)
    perm[0::2] = np.arange(64)
    perm[1::2] = 64 + np.arange(64)

    def rope_tables(cos_r, sin_r, w, extra):
        # fold norm weight (and any extra scale); sign/roll for rotate_half
        c = (cos_r * w[None, :] * extra).astype(np.float32)
        w_rot = np.roll(w, -64)
        s = (sin_r * w_rot[None, :] * extra).astype(np.float32)
        s[:, :64] *= -1.0
        return (np.ascontiguousarray(c.T[perm]),
                np.ascontiguousarray(s.T[perm]))

    idx_q = (np.arange(H)[:, None] * D + perm[None, :]).ravel()
    idx_k = (np.arange(KV)[:, None] * D + perm[None, :]).ravel()

    def split8(w):
        hi = w.astype(f8)
        lo = (w - hi.astype(np.float32)).astype(f8)
        return hi, lo

    WqT, WqrT = split8(np.ascontiguousarray(Wq.T[:, idx_q] * WSCALE))
    WkT, WkrT = split8(np.ascontiguousarray(Wk.T[:, idx_k] * WSCALE))
    WvT, WvrT = split8(np.ascontiguousarray(Wv.T * WSCALE))
    # Wo rows permuted so the kernel's paired one-shot DMA lands rows in
    # aoT's (d-partition, head-slot) feature order: row p*16+hp*2+s holds
    # feature (2hp+s)*128+p.
    rr = np.arange(H * D)
    f_order = (2 * ((rr % 16) // 2) + rr % 2) * 128 + rr // 16
    WoT, WorT = split8(np.ascontiguousarray(Wo.T[f_order] * WSCALE))

    # window-edge triangle masks for key tiles 0 and 8 of each 128-q window
    jj = np.arange(128)[:, None]
    ii = np.arange(128)[None, :]
    band = np.zeros((128, 2, 128), np.float32)
    band[:, 0, :] = np.where(jj < ii, NEG, 0.0)   # kt0: key - query < -512
    band[:, 1, :] = np.where(jj > ii, NEG, 0.0)   # kt8: key - query > +512

    in_maps = []
    for c in range(N_CORES):
        b, ch = divmod(c, 4)
        start = ch * OWN
        lo, hi = start - WIN, start + OWN + WIN
        vlo, vhi = max(lo, 0), min(hi, L)
        xh = np.zeros((HALO, HID), np.float32)
        xh[vlo - lo:vhi - lo] = x[b, vlo:vhi]
        ch_cos = np.zeros((HALO, D), np.float32)
        ch_sin = np.zeros((HALO, D), np.float32)
        ch_cos[vlo - lo:vhi - lo] = cos[vlo:vhi]
        ch_sin[vlo - lo:vhi - lo] = sin[vlo:vhi]
        ckT, skT = rope_tables(ch_cos, ch_sin, k_norm_w, 1.0)
        cqT, sqT = rope_tables(cos[start:start + OWN], sin[start:start + OWN],
                               q_norm_w, scale)
        # hi/lo fp8 split of the (transposed) halo activations
        xhT = np.ascontiguousarray(xh.T)
        x8 = xhT.astype(f8)
        xr = (xhT - x8.astype(np.float32)).astype(f8)
        # negated count of out-of-sequence keys reaching the softmax
        # denominator as exp(0)=1 (zero-padded K => score 0, unless the
        # window-edge triangle already masks that (j, i) entry)
        dcorr = np.zeros((NQT, 128), np.float32)
        for qt in range(NQT):
            q0g = start + qt * 128
            kk = q0g - WIN + np.arange(NKD * 128)
            invalid = (kk < 0) | (kk >= L)          # [1152]
            inv = invalid.reshape(NKD, 128)         # [kt, j]
            for i in range(128):
                cnt = inv.sum()
                cnt -= inv[0, :][jj[:, 0] < i].sum()   # kt0 triangle masked
                cnt -= inv[8, :][jj[:, 0] > i].sum()   # kt8 triangle masked
                dcorr[qt, i] = -float(cnt)
        in_maps.append({
            "x8T": x8, "xrT": xr,
            "WqT": WqT, "WqrT": WqrT, "WkT": WkT, "WkrT": WkrT,
            "WvT": WvT, "WvrT": WvrT, "WoT": WoT, "WorT": WorT,
            "cqT": cqT, "sqT": sqT, "ckT": ckT, "skT": skT,
            "band": band, "dcorr": dcorr.reshape(1, NQT * 128).astype(bf16),
        })
    return in_maps


def kernel(**inputs):
    _ensure_path()
    from concourse import bass_utils

    if "nc" not in _CACHE:
        _CACHE["nc"] = _build()
    nc = _CACHE["nc"]

    in_maps = _host_prep(
        np.asarray(inputs["x"]), np.asarray(inputs["cos"]),
        np.asarray(inputs["sin"]), np.asarray(inputs["Wq"]),
        np.asarray(inputs["Wk"]), np.asarray(inputs["Wv"]),
        np.asarray(inputs["Wo"]), np.asarray(inputs["q_norm_w"]),
        np.asarray(inputs["k_norm_w"]))

    res = bass_utils.run_bass_kernel_spmd(nc, in_maps,
                                          core_ids=list(range(N_CORES)))
    out = np.empty((B, L, HID), np.float32)
    for c in range(N_CORES):
        b, ch = divmod(c, 4)
        out[b, ch * OWN:(ch + 1) * OWN] = res.results[c]["out"]
    return out


# revision 44
# speedup vs baseline: 1.1814x; 1.0266x over previous
"""Trainium2 Bass kernel: sliding-window GQA attention block.

Computation (matches the PyTorch/JAX reference):
    q,k,v = x @ {Wq,Wk,Wv}.T ; QK-RMSNorm ; RoPE ; GQA repeat(4x) ;
    softmax(q k^T / sqrt(D) + sliding-window bias(|i-j|<=512)) v ; @ Wo.T

Sharding (no collectives): 8 cores = 2 batches x 4 sequence chunks of 512
tokens.  Each core computes its 512 own tokens for ALL 16 heads, using a
512-token halo either side for K/V (halo K/V recomputed locally), then the
full o_proj rows for its tokens.  Outputs concatenate on host.

Precision/speed split:
  * Q/K/V projections: fp8(e4m3) DoubleRow matmuls (0.5 cyc/row, 256-deep
    contraction) with the activation split hi+lo into TWO fp8 terms
    (x = x8 + xr) accumulated in PSUM; weights single fp8 scaled by 64
    (RMSNorm makes Q/K scale-invariant; V's 64x rides through attention).
    Net cost 2 DR units = half of fp32r, with only the W-quantization
    error (~the fp8 step of W) remaining.
  * Attention: bf16 operands (kT/qT/v/e), full-rate 1 cyc/row at any
    moving size, which enables diagonal 128-query tiling: per (head,
    128-q tile) exactly 9 key tiles are live; only tiles 0 and 8 touch
    the window edge and take a constant triangle mask; ONE grouped exp
    activation covers all 9 score tiles [128, 1152].
  * Sequence-edge (first/last core chunks): K/V and RoPE tables are
    zero-padded, so out-of-sequence keys contribute exp(0)=1 to the
    softmax denominator; a host-computed count is subtracted from the
    denominator via a tiny extra PSUM-accumulate matmul.
  * o_proj: bf16 aoT x bf16 Wo (1 cyc/row).

Layouts: projections contract over hidden with hidden on partitions; Q/K
land as [head_dim, tokens] so scores^T and PV need no transposes; RMSNorm
partition reductions via ones-matmuls; softmax denominators via
ones-matmuls accumulated alongside PV; normalization applied post-PV via
a PE broadcast of reciprocal denominators.  Head dims are host-interleaved
[0,64,1,65,...] so RoPE's rotate_half is a single DVE stream-shuffle.
"""

import numpy as np


def _ensure_path():
    try:
        import concourse  # noqa: F401
    except ImportError:
        import sys
        for p in ("/opt/trn_rl_repo", "/root/.axon_site/_ro/trn_rl_repo"):
            if p not in sys.path:
                sys.path.insert(0, p)


H, KV, D = 16, 4, 128
GQ = H // KV            # 4 query heads per kv head
WIN = 512
EPS = 1e-6
B, L, HID = 2, 2048, 2048
OWN = 512               # tokens owned per core
HALO = 1536             # key/value token window per core (own +- 512)
NKT = HALO // 128       # 12 key tiles of 128
NHK = HID // 128        # 16 contraction tiles over hidden
NHP = NHK // 2          # 8 hidden-pair tiles (256-deep fp8 DoubleRow)
NQT = OWN // 128        # 4 diagonal query tiles of 128
NKD = 9                 # key tiles per 128-query window (1024+128)/128
N_CORES = 8
WSCALE = 64.0           # host-side fp8 weight scale
NEG = -1.0e30

_CACHE = {}


def _build():
    _ensure_path()
    import concourse.mybir as mybir
    import concourse.tile as tile
    from concourse import bacc
    from contextlib import ExitStack

    F32 = mybir.dt.float32
    F32R = mybir.dt.float32r
    F8 = mybir.dt.float8e4
    BF = mybir.dt.bfloat16
    DR = mybir.MatmulPerfMode.DoubleRow
    ACTF = mybir.ActivationFunctionType

    nc = bacc.Bacc("TRN2", target_bir_lowering=False, debug=False,
                   num_devices=N_CORES)

    x8T = nc.dram_tensor("x8T", [HID, HALO], F8, kind="ExternalInput").ap()
    xrT = nc.dram_tensor("xrT", [HID, HALO], F8, kind="ExternalInput").ap()
    WqT = nc.dram_tensor("WqT", [HID, H * D], F8, kind="ExternalInput").ap()
    WqrT = nc.dram_tensor("WqrT", [HID, H * D], F8, kind="ExternalInput").ap()
    WkT = nc.dram_tensor("WkT", [HID, KV * D], F8, kind="ExternalInput").ap()
    WkrT = nc.dram_tensor("WkrT", [HID, KV * D], F8,
                          kind="ExternalInput").ap()
    WvT = nc.dram_tensor("WvT", [HID, KV * D], F8, kind="ExternalInput").ap()
    WvrT = nc.dram_tensor("WvrT", [HID, KV * D], F8,
                          kind="ExternalInput").ap()
    WoT = nc.dram_tensor("WoT", [H * D, HID], F8, kind="ExternalInput").ap()
    WorT = nc.dram_tensor("WorT", [H * D, HID], F8,
                          kind="ExternalInput").ap()
    # RoPE tables, transposed to [D, tokens], norm-weights (and for q the
    # 1/sqrt(D) score scale) folded in; s-table has rotate_half sign/roll.
    cqT = nc.dram_tensor("cqT", [D, OWN], F32, kind="ExternalInput").ap()
    sqT = nc.dram_tensor("sqT", [D, OWN], F32, kind="ExternalInput").ap()
    ckT = nc.dram_tensor("ckT", [D, HALO], F32, kind="ExternalInput").ap()
    skT = nc.dram_tensor("skT", [D, HALO], F32, kind="ExternalInput").ap()
    # [128, 2, 128] window-edge triangle masks (slot 0 -> kt0, 1 -> kt8)
    band = nc.dram_tensor("band", [128, 2, 128], F32,
                          kind="ExternalInput").ap()
    # negated out-of-sequence key counts per (q-tile, query)
    dcorr = nc.dram_tensor("dcorr", [1, NQT * 128], BF,
                           kind="ExternalInput").ap()
    out = nc.dram_tensor("out", [OWN, HID], F32, kind="ExternalOutput").ap()

    SWAP_MASK = [p ^ 1 for p in range(32)]

    with tile.TileContext(nc) as tc, ExitStack() as top:
        # ---- persistent SBUF ----
        keep = top.enter_context(tc.tile_pool(name="keep", bufs=1))
        v_sb = keep.tile([128, NKT, KV * D], BF)        # [tok128, ktile, vf]
        kT_sb = keep.tile([128, KV, HALO], BF)          # [d, kv, tok]
        qT_sb = keep.tile([128, H, OWN], BF)            # [d, h, tok]
        ones32 = keep.tile([128, 1], F32)
        nc.vector.memset(ones32, 1.0)
        ones_sb = keep.tile([128, 1], F32R)
        nc.vector.tensor_copy(ones_sb, ones32)
        ones_bf = keep.tile([128, 1], BF)
        nc.vector.tensor_copy(ones_bf, ones32)
        one1_bf = keep.tile([1, 1], BF)
        nc.vector.memset(one1_bf, 1.0)
        ones132 = keep.tile([1, 128], F32)
        nc.vector.memset(ones132, 1.0)
        ones1_sb = keep.tile([1, 128], F32R)
        nc.vector.tensor_copy(ones1_sb, ones132)
        band_sb = keep.tile([128, 2, 128], F32)
        nc.gpsimd.dma_start(out=band_sb, in_=band)
        dcorr_sb = keep.tile([1, NQT * 128], BF)
        nc.gpsimd.dma_start(out=dcorr_sb, in_=dcorr)

        # alternating resident zones: a phase's tensors prefetch while the
        # *other* zone's previous-phase readers drain
        zoneA = top.enter_context(tc.tile_pool(name="zoneA", bufs=1))
        zoneB = top.enter_context(tc.tile_pool(name="zoneB", bufs=1))
        ws = top.enter_context(tc.tile_pool(name="ws", bufs=3))

        def load_paired(dst, src, n, eng=None):
            # [256r, C] DRAM rows -> [128, n, 2, C] fp8 in ONE DMA (k = 2p+s
            # packing; any bijection works since x and W load identically)
            (eng or nc.gpsimd).dma_start(out=dst, in_=src)

        # persistent hi/lo fp8 halo activations; chunked DMAs ordered so the
        # first V-projection chunk's operands land first (the DMA's AP does
        # the [2048r, C] -> [128, 8, 2, C] pair packing)
        x8c = keep.tile([128, NHP, 2, HALO], F8)
        xrc = keep.tile([128, NHP, 2, HALO], F8)
        for ch in range(3):
            sl = slice(ch * 512, (ch + 1) * 512)
            nc.sync.dma_start(out=x8c[:, :, :, sl], in_=x8T[:, sl])
            nc.scalar.dma_start(out=xrc[:, :, :, sl], in_=xrT[:, sl])

        # ========== V projection (3-term fp8 DR: x8W8 + xrW8 + x8Wr) =====
        wv_sb = zoneA.tile([128, NHP, 2, KV * D], F8, tag="wv", name="wv_sb")
        wvr_sb = zoneA.tile([128, NHP, 2, KV * D], F8, tag="wvr",
                            name="wvr_sb")
        load_paired(wv_sb, WvT, NHP)
        load_paired(wvr_sb, WvrT, NHP)
        with ExitStack() as ph:
            ps = ph.enter_context(tc.tile_pool(name="vps", bufs=8,
                                               space="PSUM"))
            for ch in range(3):
                pv = [ps.tile([128, KV * D], F32, tag="pv", name=f"pv{t}")
                      for t in range(4)]
                c0 = ch * 512
                terms = [(x8c, wv_sb), (xrc, wv_sb), (x8c, wvr_sb)]
                for fh in range(2):
                    for t, (xc, wsb) in enumerate(terms):
                        for k in range(NHP):
                            for tt in range(4):
                                nc.tensor.matmul(
                                    pv[tt][:, fh * 256:(fh + 1) * 256],
                                    xc[:, k, :,
                                       c0 + tt * 128:c0 + (tt + 1) * 128],
                                    wsb[:, k, :, fh * 256:(fh + 1) * 256],
                                    start=(t == 0 and k == 0),
                                    stop=(t == 2 and k == NHP - 1),
                                    perf_mode=DR)
                for tt in range(4):
                    nc.scalar.copy(out=v_sb[:, ch * 4 + tt, :], in_=pv[tt])

        # ============ K / Q projection + RMSNorm + RoPE ============
        def norm_rope(p_feat, cT, sT, r_dst, n_tok, psn, scratch):
            """p_feat: psum [128 d, n_tok] raw head; writes r_dst (bf16)."""
            raw = scratch.tile([128, n_tok], F32, tag="raw")
            nc.scalar.copy(out=raw, in_=p_feat)
            sq = scratch.tile([128, n_tok], F32R, tag="sq")
            nc.vector.tensor_mul(out=sq, in0=raw, in1=raw)
            pss = psn.tile([1, n_tok], F32, tag="ss")
            nc.tensor.matmul(pss, ones_sb, sq, start=True, stop=True)
            ms = scratch.tile([1, n_tok], F32, tag="ms")
            nc.vector.tensor_scalar(out=ms, in0=pss, scalar1=1.0 / D,
                                    scalar2=EPS * WSCALE * WSCALE,
                                    op0=mybir.AluOpType.mult,
                                    op1=mybir.AluOpType.add)
            nc.vector.reciprocal(ms, ms)
            rs = scratch.tile([1, n_tok], F32R, tag="rs")
            nc.scalar.activation(out=rs, in_=ms, func=ACTF.Sqrt)
            prb = psn.tile([128, n_tok], F32, tag="rb")
            nc.tensor.matmul(prb, ones1_sb, rs, start=True, stop=True)
            swp = scratch.tile([128, n_tok], F32, tag="swp")
            nc.vector.stream_shuffle(out=swp, in_=raw, mask=SWAP_MASK)
            t1 = scratch.tile([128, n_tok], F32, tag="t1")
            nc.gpsimd.tensor_mul(out=t1, in0=raw, in1=cT)
            t2 = scratch.tile([128, n_tok], F32, tag="t2")
            nc.gpsimd.tensor_mul(out=t2, in0=swp, in1=sT)
            nc.gpsimd.tensor_add(out=t1, in0=t1, in1=t2)
            nc.vector.tensor_mul(out=r_dst, in0=t1, in1=prb)

        wk_sb = zoneB.tile([128, NHP, 2, KV * D], F8, tag="wk", name="wk_sb")
        wkr_sb = zoneB.tile([128, NHP, 2, KV * D], F8, tag="wkr",
                            name="wkr_sb")
        load_paired(wk_sb, WkT, NHP)
        load_paired(wkr_sb, WkrT, NHP)
        ck_sb = zoneB.tile([128, HALO], F32, tag="tc", name="ck_sb")
        sk_sb = zoneB.tile([128, HALO], F32, tag="ts", name="sk_sb")
        nc.gpsimd.dma_start(out=ck_sb, in_=ckT)
        nc.gpsimd.dma_start(out=sk_sb, in_=skT)

        with ExitStack() as ph:
            psp = ph.enter_context(tc.tile_pool(name="psp", bufs=6,
                                                space="PSUM"))
            psn = ph.enter_context(tc.tile_pool(name="psn", bufs=1,
                                                space="PSUM"))
            scratch = ph.enter_context(tc.tile_pool(name="scratch", bufs=2))
            for ch in range(3):
                pk = [psp.tile([128, 512], F32, tag="p", name=f"pk{t}")
                      for t in range(KV)]
                c0 = ch * 512
                terms = [(x8c, wk_sb), (xrc, wk_sb), (x8c, wkr_sb)]
                for th in range(2):
                    for t, (xc, wsb) in enumerate(terms):
                        for k in range(NHP):
                            for h in range(KV):
                                nc.tensor.matmul(
                                    pk[h][:, th * 256:(th + 1) * 256],
                                    wsb[:, k, :, h * 128:(h + 1) * 128],
                                    xc[:, k, :,
                                       c0 + th * 256:c0 + (th + 1) * 256],
                                    start=(t == 0 and k == 0),
                                    stop=(t == 2 and k == NHP - 1),
                                    perf_mode=DR)
                sl = slice(ch * 512, (ch + 1) * 512)
                for h in range(KV):
                    norm_rope(pk[h], ck_sb[:, sl], sk_sb[:, sl],
                              kT_sb[:, h, sl], 512, psn, scratch)

            # ---- Q: own-token slices of the persistent x terms ----
            cq_sb = zoneA.tile([128, OWN], F32, tag="tc", name="cq_sb")
            sq_sb = zoneA.tile([128, OWN], F32, tag="ts", name="sq_sb")
            nc.gpsimd.dma_start(out=cq_sb, in_=cqT)
            nc.gpsimd.dma_start(out=sq_sb, in_=sqT)
            for qf in range(4):
                pq = [psp.tile([128, OWN], F32, tag="p", name=f"pq{t}")
                      for t in range(4)]
                wqs, wqrs = [], []
                for src, dst in ((WqT, wqs), (WqrT, wqrs)):
                    wq = ws.tile([128, NHP, 2, 512], F8, tag="w")
                    nc.sync.dma_start(
                        out=wq, in_=src[:, qf * 512:(qf + 1) * 512])
                    dst.append(wq)
                terms = [(x8c, wqs[0]), (xrc, wqs[0]), (x8c, wqrs[0])]
                for th in range(2):
                    for t, (xc, wql) in enumerate(terms):
                        for k in range(NHP):
                            for j in range(4):
                                nc.tensor.matmul(
                                    pq[j][:, th * 256:(th + 1) * 256],
                                    wql[:, k, :, j * 128:(j + 1) * 128],
                                    xc[:, k, :,
                                       512 + th * 256:512 + (th + 1) * 256],
                                    start=(t == 0 and k == 0),
                                    stop=(t == 2 and k == NHP - 1),
                                    perf_mode=DR)
                for j in range(4):
                    norm_rope(pq[j], cq_sb, sq_sb, qT_sb[:, qf * 4 + j, :],
                              OWN, psn, scratch)

        # ================= attention (bf16, diagonal 128-q tiles) ========
        aoT8_sb = zoneA.tile([128, H, OWN], F8, tag="ao8", name="aoT8_sb")
        aoTr_sb = zoneA.tile([128, H, OWN], F8, tag="aor", name="aoTr_sb")
        with ExitStack() as ph:
            # PSUM banks: scores-A(+pb) ring 2x2 + scores-B ring 2x1
            # + po 1 + pd 1 = 8.  kt8 is scored and exp'd FIRST so PV can
            # begin while the big exp-A drains, keeping the PE fed.
            pssc = ph.enter_context(tc.tile_pool(name="aps", bufs=2,
                                                 space="PSUM"))
            pssb = ph.enter_context(tc.tile_pool(name="apsb", bufs=1,
                                                 space="PSUM"))
            pso = ph.enter_context(tc.tile_pool(name="apo", bufs=2,
                                                space="PSUM"))
            psd = ph.enter_context(tc.tile_pool(name="apd", bufs=1,
                                                space="PSUM"))
            es = ph.enter_context(tc.tile_pool(name="aes", bufs=3))
            eb = ph.enter_context(tc.tile_pool(name="aeb", bufs=3))
            sc = ph.enter_context(tc.tile_pool(name="asc", bufs=2))
            for h in range(H):
                kv = h // GQ
                po_h = pso.tile([128, OWN], F32, tag="po")
                pd_h = psd.tile([1, OWN], F32, tag="pd")
                for qt in range(NQT):
                    qsl = slice(qt * 128, (qt + 1) * 128)
                    ps_b = pssb.tile([128, 128], F32, tag="sb")
                    nc.tensor.matmul(
                        ps_b,
                        kT_sb[:, kv, (qt + 8) * 128:(qt + 9) * 128],
                        qT_sb[:, h, qsl], start=True, stop=True)
                    nc.vector.tensor_add(out=ps_b, in0=ps_b,
                                         in1=band_sb[:, 1, :])
                    e_b = eb.tile([128, 128], BF, tag="eb")
                    nc.scalar.activation(out=e_b, in_=ps_b, func=ACTF.Exp)
                    ps_s = pssc.tile([128, 8 * 128], F32, tag="s")
                    for j in range(8):
                        nc.tensor.matmul(
                            ps_s[:, j * 128:(j + 1) * 128],
                            kT_sb[:, kv, (qt + j) * 128:(qt + j + 1) * 128],
                            qT_sb[:, h, qsl], start=True, stop=True)
                        if j == 0:
                            nc.vector.tensor_add(
                                out=ps_s[:, :128], in0=ps_s[:, :128],
                                in1=band_sb[:, 0, :])
                    # two pipelined exp halves: PV for kt0..3 can start
                    # while the second half's exp is still draining
                    e = es.tile([128, 8 * 128], BF, tag="e")
                    nc.scalar.activation(out=e[:, :512], in_=ps_s[:, :512],
                                         func=ACTF.Exp)
                    nc.scalar.activation(out=e[:, 512:], in_=ps_s[:, 512:],
                                         func=ACTF.Exp)
                    nc.tensor.matmul(po_h[:, qsl],
                                     v_sb[:, qt + 8, kv * 128:(kv + 1) * 128],
                                     e_b, start=True, stop=False,
                                     skip_group_check=True)
                    nc.tensor.matmul(pd_h[:, qsl], ones_bf, e_b,
                                     start=True, stop=False,
                                     skip_group_check=True)
                    for j in range(8):
                        esl = e[:, j * 128:(j + 1) * 128]
                        nc.tensor.matmul(po_h[:, qsl],
                                         v_sb[:, qt + j,
                                              kv * 128:(kv + 1) * 128],
                                         esl, start=False, stop=(j == 7),
                                         skip_group_check=True)
                        nc.tensor.matmul(pd_h[:, qsl], ones_bf, esl,
                                         start=False, stop=False,
                                         skip_group_check=True)
                    nc.tensor.matmul(pd_h[:, qsl], one1_bf, dcorr_sb[:, qsl],
                                     start=False, stop=True,
                                     skip_group_check=True)
                dr32 = sc.tile([1, OWN], F32, tag="dr32")
                nc.vector.reciprocal(dr32, pd_h)
                drr = sc.tile([1, OWN], F32R, tag="drr")
                nc.vector.tensor_copy(drr, dr32)
                pbt = pssc.tile([128, 8 * 128], F32, tag="s", name="pbt")
                pb = pbt[:, :OWN]
                nc.tensor.matmul(pb, ones1_sb, drr, start=True, stop=True)
                bf = sc.tile([128, OWN], F32, tag="bf")
                nc.vector.tensor_copy(bf, pb)
                tf = sc.tile([128, OWN], F32, tag="tf")
                nc.vector.tensor_mul(out=tf, in0=po_h, in1=bf)
                nc.vector.tensor_copy(aoT8_sb[:, h, :], tf)
                nc.vector.tensor_tensor(out=aoTr_sb[:, h, :], in0=tf,
                                        in1=aoT8_sb[:, h, :],
                                        op=mybir.AluOpType.subtract)

        # ============ output projection (3-term fp8 DR) ============
        # Wo DRAM rows are host-permuted so a single paired DMA lands
        # [128, 8hp, 2hs, cols] matching aoT's (d, head) feature layout.
        with ExitStack() as ph:
            psy = ph.enter_context(tc.tile_pool(name="ops", bufs=8,
                                                space="PSUM"))
            ys = ph.enter_context(tc.tile_pool(name="oy", bufs=4))
            for hc in range(4):
                py = [psy.tile([128, 512], F32, tag="py", name=f"py{t}")
                      for t in range(4)]
                wo8 = ws.tile([128, H // 2, 2, 512], F8, tag="w")
                wor = ws.tile([128, H // 2, 2, 512], F8, tag="w")
                nc.sync.dma_start(out=wo8,
                                  in_=WoT[:, hc * 512:(hc + 1) * 512])
                nc.scalar.dma_start(out=wor,
                                    in_=WorT[:, hc * 512:(hc + 1) * 512])
                terms = [(aoT8_sb, wo8), (aoTr_sb, wo8), (aoT8_sb, wor)]
                for wh in range(2):
                    for t, (ao, wo) in enumerate(terms):
                        for hp in range(H // 2):
                            for tt in range(4):
                                nc.tensor.matmul(
                                    py[tt][:, wh * 256:(wh + 1) * 256],
                                    ao[:, 2 * hp:2 * hp + 2,
                                       tt * 128:(tt + 1) * 128],
                                    wo[:, hp, :, wh * 256:(wh + 1) * 256],
                                    start=(t == 0 and hp == 0),
                                    stop=(t == 2 and hp == H // 2 - 1),
                                    perf_mode=DR)
                for tt in range(4):
                    y = ys.tile([128, 512], F32, tag="y")
                    nc.vector.tensor_scalar(out=y, in0=py[tt],
                                            scalar1=1.0 / (WSCALE * WSCALE),
                                            scalar2=0.0,
                                            op0=mybir.AluOpType.mult,
                                            op1=mybir.AluOpType.add)
                    nc.sync.dma_start(
                        out=out[tt * 128:(tt + 1) * 128,
                                hc * 512:(hc + 1) * 512], in_=y)

    nc.compile()
    return nc


def _host_prep(x, cos, sin, Wq, Wk, Wv, Wo, q_norm_w, k_norm_w):
    """Build the 8 per-core input dicts."""
    import ml_dtypes
    f8 = ml_dtypes.float8_e4m3
    bf16 = ml_dtypes.bfloat16
    scale = 1.0 / np.sqrt(D)
    # interleave head dims [0,64,1,65,...]: rotate_half partners end up on
    # adjacent partitions so the kernel swaps them with one stream-shuffle
    perm = np.empty(D, np.int64)
    perm[0::2] = np.arange(64)
    perm[1::2] = 64 + np.arange(64)

    def rope_tables(cos_r, sin_r, w, extra):
        # fold norm weight (and any extra scale); sign/roll for rotate_half
        c = (cos_r * w[None, :] * extra).astype(np.float32)
        w_rot = np.roll(w, -64)
        s = (sin_r * w_rot[None, :] * extra).astype(np.float32)
        s[:, :64] *= -1.0
        return (np.ascontiguousarray(c.T[perm]),
                np.ascontiguousarray(s.T[perm]))

    idx_q = (np.arange(H)[:, None] * D + perm[None, :]).ravel()
    idx_k = (np.arange(KV)[:, None] * D + perm[None, :]).ravel()

    def split8(w):
        hi = w.astype(f8)
        lo = (w - hi.astype(np.float32)).astype(f8)
        return hi, lo

    WqT, WqrT = split8(np.ascontiguousarray(Wq.T[:, idx_q] * WSCALE))
    WkT, WkrT = split8(np.ascontiguousarray(Wk.T[:, idx_k] * WSCALE))
    WvT, WvrT = split8(np.ascontiguousarray(Wv.T * WSCALE))
    # Wo rows permuted so the kernel's paired one-shot DMA lands rows in
    # aoT's (d-partition, head-slot) feature order: row p*16+hp*2+s holds
    # feature (2hp+s)*128+p.
    rr = np.arange(H * D)
    f_order = (2 * ((rr % 16) // 2) + rr % 2) * 128 + rr // 16
    WoT, WorT = split8(np.ascontiguousarray(Wo.T[f_order] * WSCALE))

    # window-edge triangle masks for key tiles 0 and 8 of each 128-q window
    jj = np.arange(128)[:, None]
    ii = np.arange(128)[None, :]
    band = np.zeros((128, 2, 128), np.float32)
    band[:, 0, :] = np.where(jj < ii, NEG, 0.0)   # kt0: key - query < -512
    band[:, 1, :] = np.where(jj > ii, NEG, 0.0)   # kt8: key - query > +512

    in_maps = []
    for c in range(N_CORES):
        b, ch = divmod(c, 4)
        start = ch * OWN
        lo, hi = start - WIN, start + OWN + WIN
        vlo, vhi = max(lo, 0), min(hi, L)
        xh = np.zeros((HALO, HID), np.float32)
        xh[vlo - lo:vhi - lo] = x[b, vlo:vhi]
        ch_cos = np.zeros((HALO, D), np.float32)
        ch_sin = np.zeros((HALO, D), np.float32)
        ch_cos[vlo - lo:vhi - lo] = cos[vlo:vhi]
        ch_sin[vlo - lo:vhi - lo] = sin[vlo:vhi]
        ckT, skT = rope_tables(ch_cos, ch_sin, k_norm_w, 1.0)
        cqT, sqT = rope_tables(cos[start:start + OWN], sin[start:start + OWN],
                               q_norm_w, scale)
        # hi/lo fp8 split of the (transposed) halo activations
        xhT = np.ascontiguousarray(xh.T)
        x8 = xhT.astype(f8)
        xr = (xhT - x8.astype(np.float32)).astype(f8)
        # negated count of out-of-sequence keys reaching the softmax
        # denominator as exp(0)=1 (zero-padded K => score 0, unless the
        # window-edge triangle already masks that (j, i) entry)
        dcorr = np.zeros((NQT, 128), np.float32)
        for qt in range(NQT):
            q0g = start + qt * 128
            kk = q0g - WIN + np.arange(NKD * 128)
            invalid = (kk < 0) | (kk >= L)          # [1152]
            inv = invalid.reshape(NKD, 128)         # [kt, j]
            for i in range(128):
                cnt = inv.sum()
                cnt -= inv[0, :][jj[:, 0] < i].sum()   # kt0 triangle masked
                cnt -= inv[8, :][jj[:, 0] > i].sum()   # kt8 triangle masked
                dcorr[qt, i] = -float(cnt)
        in_maps.append({
            "x8T": x8, "xrT": xr,
            "WqT": WqT, "WqrT": WqrT, "WkT": WkT, "WkrT": WkrT,
            "WvT": WvT, "WvrT": WvrT, "WoT": WoT, "WorT": WorT,
            "cqT": cqT, "sqT": sqT, "ckT": ckT, "skT": skT,
            "band": band, "dcorr": dcorr.reshape(1, NQT * 128).astype(bf16),
        })
    return in_maps


def kernel(**inputs):
    _ensure_path()
    from concourse import bass_utils

    if "nc" not in _CACHE:
        _CACHE["nc"] = _build()
    nc = _CACHE["nc"]

    in_maps = _host_prep(
        np.asarray(inputs["x"]), np.asarray(inputs["cos"]),
        np.asarray(inputs["sin"]), np.asarray(inputs["Wq"]),
        np.asarray(inputs["Wk"]), np.asarray(inputs["Wv"]),
        np.asarray(inputs["Wo"]), np.asarray(inputs["q_norm_w"]),
        np.asarray(inputs["k_norm_w"]))

    res = bass_utils.run_bass_kernel_spmd(nc, in_maps,
                                          core_ids=list(range(N_CORES)))
    out = np.empty((B, L, HID), np.float32)
    for c in range(N_CORES):
        b, ch = divmod(c, 4)
        out[b, ch * OWN:(ch + 1) * OWN] = res.results[c]["out"]
    return out
